# revision 5
# baseline (speedup 1.0000x reference)
"""GraphSAGE conv layer (PyG SAGEConv, aggr='mean') on 8 Trainium2 NeuronCores.

    out = relu(mean_j(x[src_j]) @ W_l + b_l + x @ W_r)

Sharding: edges are partitioned by destination node across the 8 cores (6250
destination nodes per core); the small 128x128 weights are replicated; each
core keeps a full copy of x in its DRAM so the per-edge source-feature gather
stays local (full-input replication instead of a halo exchange).

Per-core device pipeline (bf16 data, f32 accumulation):
  - Edges are bucketed host-side by (512-node destination group, source-index
    window) and fetched edge-major with bulk `dma_gather` ops. Gather calls
    are striped across all 4 SWDGE queues so all four Q7 core pairs generate
    DMA descriptors concurrently (descriptor generation is the dominant cost).
  - Per-core padding slack in the shared-size index tables is -1, which the
    Q7 gather kernel skips entirely (no descriptors, no DMA).
  - For each 128-edge column the DVE builds a scaled one-hot selector in bf16
    (is_equal against an iota, times 1/deg of the destination) and the PE
    contracts messages^T @ onehot into PSUM, accumulating the feature-major
    per-node mean directly (segment mean == one matmul chain per node tile).
  - Weight-stationary bf16 matmuls add W_l.T @ meanT + W_r.T @ xT, ACT fuses
    bias + ReLU, and the result is stored feature-major; the host transposes
    while assembling the full output.
"""

import math

import numpy as np

N_CORES = 8
D = 128
P = 128
TPG = 4           # node tiles (of 128 nodes) per PSUM group -> 512 wide
WINDOW_ROWS = 32768   # dma_gather int16 index window (rows)
N_QUEUES = 1          # SWDGE queues (Q7 core pairs) to stripe gathers across


def _bf16(a):
    import ml_dtypes
    return np.ascontiguousarray(np.asarray(a).astype(ml_dtypes.bfloat16))


# ---------------------------------------------------------------------------
# Host-side sharding / table prep
# ---------------------------------------------------------------------------

def _prep(x, src, dst, n_cores):
    n, d = x.shape
    assert d == D
    npc = n // n_cores
    assert npc * n_cores == n
    t0 = math.ceil(npc / P)          # node tiles per core
    g0 = math.ceil(t0 / TPG)         # groups per core
    nrank = t0 * P

    # source windows over the device x copy: row 0 and row n+1 are zeros,
    # x row i lives at device row i+1 (50002 rows total).
    n_dev = n + 2
    assert 2 * WINDOW_ROWS >= n_dev, "two windows must cover all of x"
    a_max_src = min(WINDOW_ROWS - 1, n_dev - 1) - 1   # src s -> row s+1
    b_base = max(0, n_dev - WINDOW_ROWS)
    b_pad = min(n_dev - 1, WINDOW_ROWS - 1)           # window-local zero row

    per_core = []
    cnt = np.zeros((n_cores, g0, 2), dtype=np.int64)
    # per (core, g, w, t_loc): first/last edge position inside the call
    starts3 = np.full((n_cores, g0, 2, TPG), -1, dtype=np.int64)
    ends3 = np.full((n_cores, g0, 2, TPG), -1, dtype=np.int64)

    for m in range(n_cores):
        sel = (dst >= m * npc) & (dst < (m + 1) * npc)
        s = src[sel]
        dl = dst[sel] - m * npc
        deg = np.bincount(dl, minlength=npc)
        recip_node = (1.0 / np.maximum(deg, 1)).astype(np.float32)
        tile = dl // P
        grp = tile // TPG
        w = (s > a_max_src).astype(np.int64)
        order = np.lexsort((tile, w, grp))
        s, dl, tile, grp, w = s[order], dl[order], tile[order], grp[order], w[order]
        t_loc = tile - grp * TPG

        for g in range(g0):
            for wi in range(2):
                selgw = (grp == g) & (w == wi)
                cnt[m, g, wi] = selgw.sum()
                if cnt[m, g, wi] == 0:
                    continue
                base = np.nonzero(selgw)[0][0]
                for tl in range(TPG):
                    st = (grp == g) & (w == wi) & (t_loc == tl)
                    c = st.sum()
                    if c == 0:
                        continue
                    first = np.nonzero(st)[0][0] - base
                    starts3[m, g, wi, tl] = first
                    ends3[m, g, wi, tl] = first + c
        per_core.append((s, dl, grp, w, t_loc, recip_node))

    # shared call sizes (in 128-edge columns)
    S = np.ceil(cnt / P).astype(np.int64).max(axis=0)     # [g0, 2]
    colstart = np.zeros((g0, 2), dtype=np.int64)          # per-window cumulative
    acc = [0, 0]
    for g in range(g0):
        for wi in range(2):
            colstart[g, wi] = acc[wi]
            acc[wi] += S[g, wi]
    tot_cols = (acc[0], acc[1])

    # stripe gather calls across SWDGE queues, greedy-balanced by size
    qloads = [0] * N_QUEUES
    queue_of = np.zeros((g0, 2), dtype=np.int64)
    for g in range(g0):
        for wi in range(2):
            if S[g, wi] == 0:
                continue
            q = min(range(N_QUEUES), key=lambda i: qloads[i])
            queue_of[g, wi] = q
            qloads[q] += int(S[g, wi])

    # shared use lists: per (g, w, t_loc) the union (over cores) column range
    uses = [[] for _ in range(g0)]    # per group: list of (w, c, t_loc)
    use_c0 = np.full((g0, 2, TPG), -1, dtype=np.int64)
    use_u0 = np.full((g0, 2, TPG), -1, dtype=np.int64)
    u_tot = 0
    for g in range(g0):
        n_tiles = min(TPG, t0 - g * TPG)
        for tl in range(n_tiles):
            tile_uses = []
            for wi in range(2):
                stm = starts3[:, g, wi, tl]
                enm = ends3[:, g, wi, tl]
                anyc = stm >= 0
                if not anyc.any():
                    continue
                c_lo = int((stm[anyc] // P).min())
                c_hi = int(((enm[anyc] - 1) // P).max())
                use_c0[g, wi, tl] = c_lo
                use_u0[g, wi, tl] = u_tot + len(tile_uses)
                for c in range(c_lo, c_hi + 1):
                    tile_uses.append((wi, c, tl))
            if not tile_uses:
                # keep the PSUM slice defined: one all-masked use
                wi = 0 if S[g, 0] > 0 else 1
                assert S[g, wi] > 0, f"group {g} has no gather columns at all"
                tile_uses.append((wi, 0, tl))
            uses[g].extend(tile_uses)
    u_tot = sum(len(u) for u in uses)

    # global u index per (g, position-in-group-list)
    u_base = np.zeros(g0, dtype=np.int64)
    accu = 0
    for g in range(g0):
        u_base[g] = accu
        accu += len(uses[g])

    # map (g, w, t_loc, c) -> global u (uses within a tile are consecutive cols)
    u_lookup = {}
    for g in range(g0):
        for pos, (wi, c, tl) in enumerate(uses[g]):
            u_lookup[(g, wi, tl, c)] = u_base[g] + pos

    in_parts = []
    for m in range(n_cores):
        s, dl, grp, w, t_loc, recip_node = per_core[m]
        idx_t = [
            np.zeros(int(tot_cols[0]) * P, dtype=np.int16),
            np.full(int(tot_cols[1]) * P, b_pad, dtype=np.int16),
        ]
        dstp = np.full((P, u_tot), -1.0, dtype=np.float32)
        recipe = np.zeros((P, u_tot), dtype=np.float32)

        # per-edge position j inside its (g, w) call
        call_of_edge = grp * 2 + w
        call_sizes = np.bincount(call_of_edge, minlength=g0 * 2)
        call_first = np.concatenate([[0], np.cumsum(call_sizes)])[:-1]
        j = np.arange(len(s)) - call_first[call_of_edge]
        col = j // P
        p = j % P

        # gather index values
        lin = (colstart[grp, w] * P + j).astype(np.int64)
        val_a = (s + 1).astype(np.int64)
        val_b = (s + 1 - b_base).astype(np.int64)
        isa = w == 0
        idx_t[0][lin[isa]] = val_a[isa]
        idx_t[1][lin[~isa]] = val_b[~isa]

        # dstloc / recip tables per use
        u_edge = np.empty(len(s), dtype=np.int64)
        for g in range(g0):
            for wi in range(2):
                for tl in range(TPG):
                    selgwt = (grp == g) & (w == wi) & (t_loc == tl)
                    if not selgwt.any():
                        continue
                    u0 = u_lookup[(g, wi, tl, int(use_c0[g, wi, tl]))]
                    c0 = use_c0[g, wi, tl]
                    u_edge[selgwt] = u0 + (col[selgwt] - c0)
        dstp[p, u_edge] = (dl - (grp * TPG + t_loc) * P).astype(np.float32)
        recipe[p, u_edge] = recip_node[dl]

        # wrap idx tables to [128, n/16]: linear idx i lives at
        # [i % 16, i // 16], replicated 8x down the partition dim (one copy
        # per Q7 core pair).
        def wrap(a):
            if len(a) == 0:
                return np.zeros((128, 0), dtype=np.int16)
            w16 = a.reshape(-1, 16).T
            return np.ascontiguousarray(np.tile(w16, (8, 1)))

        xt = np.zeros((P, nrank), dtype=np.float32)
        xt[:, :npc] = x[m * npc:(m + 1) * npc].T

        in_parts.append({
            "idxa": wrap(idx_t[0]),
            "idxb": wrap(idx_t[1]),
            "dstp": dstp,
            "recipe": recipe,
            "xt": np.ascontiguousarray(xt),
        })

    meta = {
        "n": n, "npc": npc, "t0": t0, "g0": g0, "nrank": nrank,
        "S": S, "colstart": colstart, "tot_cols": tot_cols,
        "uses": uses, "u_tot": u_tot, "b_base": b_base,
        "queue_of": queue_of,
    }
    return meta, in_parts


# ---------------------------------------------------------------------------
# Device kernel builder
# ---------------------------------------------------------------------------

def _build(meta):
    from contextlib import ExitStack

    import concourse.bass as bass  # noqa: F401
    import concourse.mybir as mybir
    import concourse.tile as tile
    from concourse import bacc

    f32 = mybir.dt.float32
    bf16 = mybir.dt.bfloat16
    i16 = mybir.dt.int16
    i32 = mybir.dt.int32
    n = meta["n"]
    t0 = meta["t0"]
    g0 = meta["g0"]
    nrank = meta["nrank"]
    S = meta["S"]
    colstart = meta["colstart"]
    tot_a, tot_b = meta["tot_cols"]
    uses = meta["uses"]
    u_tot = meta["u_tot"]
    b_base = meta["b_base"]
    queue_of = meta["queue_of"]
    n_dev = n + 2
    win_a = min(WINDOW_ROWS, n_dev)
    max_sa = int(max((S[g, 0] for g in range(g0)), default=1)) or 1
    max_sb = int(max((S[g, 1] for g in range(g0)), default=1)) or 1

    nc = bacc.Bacc("TRN2", target_bir_lowering=False)
    x2_d = nc.dram_tensor("xrows", [n_dev, D], f32, kind="ExternalInput")
    idxa_d = nc.dram_tensor("idxa", [P, max(tot_a * 8, 1)], i16, kind="ExternalInput")
    idxb_d = nc.dram_tensor("idxb", [P, max(tot_b * 8, 1)], i16, kind="ExternalInput")
    dstp_d = nc.dram_tensor("dstp", [P, u_tot], f32, kind="ExternalInput")
    recipe_d = nc.dram_tensor("recipe", [P, u_tot], f32, kind="ExternalInput")
    xt_d = nc.dram_tensor("xt", [P, nrank], f32, kind="ExternalInput")
    wl_d = nc.dram_tensor("wl", [D, D], f32, kind="ExternalInput")
    wr_d = nc.dram_tensor("wr", [D, D], f32, kind="ExternalInput")
    b_d = nc.dram_tensor("bias", [D, 1], f32, kind="ExternalInput")
    out_d = nc.dram_tensor("outT", [P, nrank], f32, kind="ExternalOutput")

    with ExitStack() as ctx:
        tc = ctx.enter_context(tile.TileContext(nc))
        const = ctx.enter_context(tc.tile_pool(name="const", bufs=1))
        stga_pool = ctx.enter_context(tc.tile_pool(name="stga", bufs=2))
        stgb_pool = ctx.enter_context(tc.tile_pool(name="stgb", bufs=2))
        oh_pool = ctx.enter_context(tc.tile_pool(name="oh", bufs=6))
        mt_pool = ctx.enter_context(tc.tile_pool(name="mt", bufs=2))
        xt_pool = ctx.enter_context(tc.tile_pool(name="xtp", bufs=2))
        out_pool = ctx.enter_context(tc.tile_pool(name="outp", bufs=2))
        mt_psum = ctx.enter_context(tc.tile_pool(name="mtps", bufs=2, space="PSUM"))
        z_psum = ctx.enter_context(tc.tile_pool(name="zps", bufs=2, space="PSUM"))

        iota_i = const.tile([P, D], i32)
        nc.gpsimd.iota(iota_i[:], pattern=[[1, D]], base=0, channel_multiplier=0)
        iota_f = const.tile([P, D], f32)
        nc.vector.tensor_copy(iota_f[:], iota_i[:])

        idxa_sb = const.tile([P, max(tot_a * 8, 1)], i16)
        nc.sync.dma_start(idxa_sb[:], idxa_d[:, :])
        idxb_sb = const.tile([P, max(tot_b * 8, 1)], i16)
        nc.sync.dma_start(idxb_sb[:], idxb_d[:, :])
        dstp_sb = const.tile([P, u_tot], f32)
        nc.sync.dma_start(dstp_sb[:], dstp_d[:, :])
        recipe_sb = const.tile([P, u_tot], f32)
        nc.sync.dma_start(recipe_sb[:], recipe_d[:, :])
        wl_sb = const.tile([D, D], f32)
        nc.sync.dma_start(wl_sb[:], wl_d[:, :])
        wr_sb = const.tile([D, D], f32)
        nc.sync.dma_start(wr_sb[:], wr_d[:, :])
        b_sb = const.tile([D, 1], f32)
        nc.sync.dma_start(b_sb[:], b_d[:, :])

        u_run = 0
        for g in range(g0):
            n_tiles = min(TPG, t0 - g * TPG)
            sa, sb = int(S[g, 0]), int(S[g, 1])
            stg = [None, None]
            if sa > 0:
                stg[0] = stga_pool.tile([P, max_sa * D], f32, tag="stga", name=f"stga_{g}")
                nc.gpsimd.dma_gather(
                    out_ap=stg[0][:, :sa * D].rearrange("p (s e) -> p s e", e=D),
                    in_ap=x2_d[0:win_a, :],
                    idxs_ap=idxa_sb[:, colstart[g, 0] * 8:(colstart[g, 0] + sa) * 8],
                    num_idxs=sa * P,
                    num_idxs_reg=sa * P,
                    elem_size=D,
                    single_packet=False,
                )
            if sb > 0:
                stg[1] = stgb_pool.tile([P, max_sb * D], f32, tag="stgb", name=f"stgb_{g}")
                nc.gpsimd.dma_gather(
                    out_ap=stg[1][:, :sb * D].rearrange("p (s e) -> p s e", e=D),
                    in_ap=x2_d[b_base:n_dev, :],
                    idxs_ap=idxb_sb[:, colstart[g, 1] * 8:(colstart[g, 1] + sb) * 8],
                    num_idxs=sb * P,
                    num_idxs_reg=sb * P,
                    elem_size=D,
                    single_packet=False,
                )

            xt_sb = xt_pool.tile([P, TPG * D], f32, tag="xt")
            nc.sync.dma_start(
                xt_sb[:, :n_tiles * D],
                xt_d[:, g * TPG * D:(g * TPG + n_tiles) * D],
            )

            mt_ps = mt_psum.tile([P, TPG * D], f32, space="PSUM")
            glist = uses[g]
            # first/last use index per tile for start/stop flags
            first_of = {}
            last_of = {}
            for pos, (wi, c, tl) in enumerate(glist):
                first_of.setdefault(tl, pos)
                last_of[tl] = pos
            for pos, (wi, c, tl) in enumerate(glist):
                oh = oh_pool.tile([P, D], f32, tag="oh")
                uu = u_run + pos
                nc.vector.tensor_scalar(
                    out=oh[:],
                    in0=iota_f[:],
                    scalar1=dstp_sb[:, uu:uu + 1],
                    scalar2=recipe_sb[:, uu:uu + 1],
                    op0=mybir.AluOpType.is_equal,
                    op1=mybir.AluOpType.mult,
                )
                nc.tensor.matmul(
                    out=mt_ps[:, tl * D:(tl + 1) * D],
                    lhsT=stg[wi][:, c * D:(c + 1) * D],
                    rhs=oh[:],
                    start=(pos == first_of[tl]),
                    stop=(pos == last_of[tl]),
                )
            u_run += len(glist)

            mt_sb = mt_pool.tile([P, TPG * D], f32, tag="mt")
            nc.scalar.copy(mt_sb[:, :n_tiles * D], mt_ps[:, :n_tiles * D])
            z_ps = z_psum.tile([P, TPG * D], f32, space="PSUM")
            nc.tensor.matmul(out=z_ps[:, :n_tiles * D], lhsT=wl_sb[:],
                             rhs=mt_sb[:, :n_tiles * D], start=True, stop=False)
            nc.tensor.matmul(out=z_ps[:, :n_tiles * D], lhsT=wr_sb[:],
                             rhs=xt_sb[:, :n_tiles * D], start=False, stop=True)
            o_sb = out_pool.tile([P, TPG * D], f32, tag="o")
            nc.scalar.activation(
                o_sb[:, :n_tiles * D], z_ps[:, :n_tiles * D],
                mybir.ActivationFunctionType.Relu, bias=b_sb[:, :1], scale=1.0,
            )
            nc.sync.dma_start(
                out_d[:, g * TPG * D:(g * TPG + n_tiles) * D],
                o_sb[:, :n_tiles * D],
            )

    nc.compile()
    return nc


# ---------------------------------------------------------------------------
# Top level
# ---------------------------------------------------------------------------

def _run(inputs, trace=False):
    from concourse import bass_utils

    x = np.ascontiguousarray(np.asarray(inputs["x"], dtype=np.float32))
    ei = np.asarray(inputs["edge_index"], dtype=np.int64)
    w_l = np.asarray(inputs["W_l"], dtype=np.float32)
    b_l = np.ascontiguousarray(np.asarray(inputs["b_l"], dtype=np.float32))
    w_r = np.asarray(inputs["W_r"], dtype=np.float32)
    src, dst = ei[0], ei[1]

    meta, in_parts = _prep(x, src, dst, N_CORES)
    nc = _build(meta)

    n = meta["n"]
    xrows = np.zeros((n + 2, D), dtype=np.float32)
    xrows[1:n + 1] = x
    xrows_bf = xrows
    wl_bf = np.ascontiguousarray(w_l)
    wr_bf = np.ascontiguousarray(w_r)
    b_col = np.ascontiguousarray(b_l.reshape(D, 1), dtype=np.float32)
    in_maps = []
    for m in range(N_CORES):
        part = in_parts[m]
        in_maps.append({
            "xrows": xrows_bf,
            "idxa": _pad_cols(part["idxa"]),
            "idxb": _pad_cols(part["idxb"]),
            "dstp": part["dstp"],
            "recipe": part["recipe"],
            "xt": part["xt"],
            "wl": wl_bf,
            "wr": wr_bf,
            "bias": b_col,
        })

    results = bass_utils.run_bass_kernel_spmd(
        nc, in_maps, core_ids=list(range(N_CORES)), trace=trace
    )

    npc = meta["npc"]
    out = np.empty((n, D), dtype=np.float32)
    for m in range(N_CORES):
        out_t = results.results[m]["outT"]  # [128, nrank] feature-major
        out[m * npc:(m + 1) * npc] = out_t[:, :npc].T
    return out, results


def _pad_cols(a):
    """int16 idx tables can be [128, 0]; the dram tensor is [128, >=1]."""
    if a.shape[1] == 0:
        return np.zeros((128, 1), dtype=np.int16)
    return a


def kernel(**inputs) -> np.ndarray:
    return _run(inputs)[0]


# revision 6
# speedup vs baseline: 1.1176x; 1.1176x over previous
"""GraphSAGE conv layer (PyG SAGEConv, aggr='mean') on 8 Trainium2 NeuronCores.

    out = relu(mean_j(x[src_j]) @ W_l + b_l + x @ W_r)

Sharding: edges are partitioned by destination node across the 8 cores (6250
destination nodes per core); the small 128x128 weights are replicated; each
core keeps a full copy of x in its DRAM so the per-edge source-feature gather
stays local (full-input replication instead of a halo exchange).

Per-core device pipeline (bf16 data, f32 accumulation):
  - Edges are bucketed host-side by (512-node destination group, source-index
    window) and fetched edge-major with bulk `dma_gather` ops. Gather calls
    are striped across all 4 SWDGE queues so all four Q7 core pairs generate
    DMA descriptors concurrently (descriptor generation is the dominant cost).
  - Per-core padding slack in the shared-size index tables is -1, which the
    Q7 gather kernel skips entirely (no descriptors, no DMA).
  - For each 128-edge column the DVE builds a scaled one-hot selector in bf16
    (is_equal against an iota, times 1/deg of the destination) and the PE
    contracts messages^T @ onehot into PSUM, accumulating the feature-major
    per-node mean directly (segment mean == one matmul chain per node tile).
  - Weight-stationary bf16 matmuls add W_l.T @ meanT + W_r.T @ xT, ACT fuses
    bias + ReLU, and the result is stored feature-major; the host transposes
    while assembling the full output.
"""

import math

import numpy as np

N_CORES = 8
D = 128
P = 128
TPG = 4           # node tiles (of 128 nodes) per PSUM group -> 512 wide
WINDOW_ROWS = 32768   # dma_gather int16 index window (rows)
N_QUEUES = 1          # SWDGE queues (Q7 core pairs) to stripe gathers across


def _bf16(a):
    import ml_dtypes
    return np.ascontiguousarray(np.asarray(a).astype(ml_dtypes.bfloat16))


# ---------------------------------------------------------------------------
# Host-side sharding / table prep
# ---------------------------------------------------------------------------

def _prep(x, src, dst, n_cores):
    n, d = x.shape
    assert d == D
    npc = n // n_cores
    assert npc * n_cores == n
    t0 = math.ceil(npc / P)          # node tiles per core
    g0 = math.ceil(t0 / TPG)         # groups per core
    nrank = t0 * P

    # source windows over the device x copy: row 0 and row n+1 are zeros,
    # x row i lives at device row i+1 (50002 rows total).
    n_dev = n + 2
    assert 2 * WINDOW_ROWS >= n_dev, "two windows must cover all of x"
    a_max_src = min(WINDOW_ROWS - 1, n_dev - 1) - 1   # src s -> row s+1
    b_base = max(0, n_dev - WINDOW_ROWS)
    b_pad = min(n_dev - 1, WINDOW_ROWS - 1)           # window-local zero row

    per_core = []
    cnt = np.zeros((n_cores, g0, 2), dtype=np.int64)
    # per (core, g, w, t_loc): first/last edge position inside the call
    starts3 = np.full((n_cores, g0, 2, TPG), -1, dtype=np.int64)
    ends3 = np.full((n_cores, g0, 2, TPG), -1, dtype=np.int64)

    for m in range(n_cores):
        sel = (dst >= m * npc) & (dst < (m + 1) * npc)
        s = src[sel]
        dl = dst[sel] - m * npc
        deg = np.bincount(dl, minlength=npc)
        recip_node = (1.0 / np.maximum(deg, 1)).astype(np.float32)
        tile = dl // P
        grp = tile // TPG
        w = (s > a_max_src).astype(np.int64)
        order = np.lexsort((tile, w, grp))
        s, dl, tile, grp, w = s[order], dl[order], tile[order], grp[order], w[order]
        t_loc = tile - grp * TPG

        for g in range(g0):
            for wi in range(2):
                selgw = (grp == g) & (w == wi)
                cnt[m, g, wi] = selgw.sum()
                if cnt[m, g, wi] == 0:
                    continue
                base = np.nonzero(selgw)[0][0]
                for tl in range(TPG):
                    st = (grp == g) & (w == wi) & (t_loc == tl)
                    c = st.sum()
                    if c == 0:
                        continue
                    first = np.nonzero(st)[0][0] - base
                    starts3[m, g, wi, tl] = first
                    ends3[m, g, wi, tl] = first + c
        per_core.append((s, dl, grp, w, t_loc, recip_node))

    # shared call sizes (in 128-edge columns)
    S = np.ceil(cnt / P).astype(np.int64).max(axis=0)     # [g0, 2]
    colstart = np.zeros((g0, 2), dtype=np.int64)          # per-window cumulative
    acc = [0, 0]
    for g in range(g0):
        for wi in range(2):
            colstart[g, wi] = acc[wi]
            acc[wi] += S[g, wi]
    tot_cols = (acc[0], acc[1])

    # stripe gather calls across SWDGE queues, greedy-balanced by size
    qloads = [0] * N_QUEUES
    queue_of = np.zeros((g0, 2), dtype=np.int64)
    for g in range(g0):
        for wi in range(2):
            if S[g, wi] == 0:
                continue
            q = min(range(N_QUEUES), key=lambda i: qloads[i])
            queue_of[g, wi] = q
            qloads[q] += int(S[g, wi])

    # shared use lists: per (g, w, t_loc) the union (over cores) column range
    uses = [[] for _ in range(g0)]    # per group: list of (w, c, t_loc)
    use_c0 = np.full((g0, 2, TPG), -1, dtype=np.int64)
    use_u0 = np.full((g0, 2, TPG), -1, dtype=np.int64)
    u_tot = 0
    for g in range(g0):
        n_tiles = min(TPG, t0 - g * TPG)
        for tl in range(n_tiles):
            tile_uses = []
            for wi in range(2):
                stm = starts3[:, g, wi, tl]
                enm = ends3[:, g, wi, tl]
                anyc = stm >= 0
                if not anyc.any():
                    continue
                c_lo = int((stm[anyc] // P).min())
                c_hi = int(((enm[anyc] - 1) // P).max())
                use_c0[g, wi, tl] = c_lo
                use_u0[g, wi, tl] = u_tot + len(tile_uses)
                for c in range(c_lo, c_hi + 1):
                    tile_uses.append((wi, c, tl))
            if not tile_uses:
                # keep the PSUM slice defined: one all-masked use
                wi = 0 if S[g, 0] > 0 else 1
                assert S[g, wi] > 0, f"group {g} has no gather columns at all"
                tile_uses.append((wi, 0, tl))
            uses[g].extend(tile_uses)
    u_tot = sum(len(u) for u in uses)

    # global u index per (g, position-in-group-list)
    u_base = np.zeros(g0, dtype=np.int64)
    accu = 0
    for g in range(g0):
        u_base[g] = accu
        accu += len(uses[g])

    # map (g, w, t_loc, c) -> global u (uses within a tile are consecutive cols)
    u_lookup = {}
    for g in range(g0):
        for pos, (wi, c, tl) in enumerate(uses[g]):
            u_lookup[(g, wi, tl, c)] = u_base[g] + pos

    in_parts = []
    for m in range(n_cores):
        s, dl, grp, w, t_loc, recip_node = per_core[m]
        idx_t = [
            np.zeros(int(tot_cols[0]) * P, dtype=np.int16),
            np.full(int(tot_cols[1]) * P, b_pad, dtype=np.int16),
        ]
        dstp = np.full((P, u_tot), -1.0, dtype=np.float32)
        recipe = np.zeros((P, u_tot), dtype=np.float32)

        # per-edge position j inside its (g, w) call
        call_of_edge = grp * 2 + w
        call_sizes = np.bincount(call_of_edge, minlength=g0 * 2)
        call_first = np.concatenate([[0], np.cumsum(call_sizes)])[:-1]
        j = np.arange(len(s)) - call_first[call_of_edge]
        col = j // P
        p = j % P

        # gather index values
        lin = (colstart[grp, w] * P + j).astype(np.int64)
        val_a = (s + 1).astype(np.int64)
        val_b = (s + 1 - b_base).astype(np.int64)
        isa = w == 0
        idx_t[0][lin[isa]] = val_a[isa]
        idx_t[1][lin[~isa]] = val_b[~isa]

        # dstloc / recip tables per use
        u_edge = np.empty(len(s), dtype=np.int64)
        for g in range(g0):
            for wi in range(2):
                for tl in range(TPG):
                    selgwt = (grp == g) & (w == wi) & (t_loc == tl)
                    if not selgwt.any():
                        continue
                    u0 = u_lookup[(g, wi, tl, int(use_c0[g, wi, tl]))]
                    c0 = use_c0[g, wi, tl]
                    u_edge[selgwt] = u0 + (col[selgwt] - c0)
        dstp[p, u_edge] = (dl - (grp * TPG + t_loc) * P).astype(np.float32)
        recipe[p, u_edge] = recip_node[dl]

        # wrap idx tables to [128, n/16]: linear idx i lives at
        # [i % 16, i // 16], replicated 8x down the partition dim (one copy
        # per Q7 core pair).
        def wrap(a):
            if len(a) == 0:
                return np.zeros((128, 0), dtype=np.int16)
            w16 = a.reshape(-1, 16).T
            return np.ascontiguousarray(np.tile(w16, (8, 1)))

        xt = np.zeros((P, nrank), dtype=np.float32)
        xt[:, :npc] = x[m * npc:(m + 1) * npc].T

        in_parts.append({
            "idxa": wrap(idx_t[0]),
            "idxb": wrap(idx_t[1]),
            "dstp": dstp,
            "recipe": recipe,
            "xt": _bf16(xt),
        })

    meta = {
        "n": n, "npc": npc, "t0": t0, "g0": g0, "nrank": nrank,
        "S": S, "colstart": colstart, "tot_cols": tot_cols,
        "uses": uses, "u_tot": u_tot, "b_base": b_base,
        "queue_of": queue_of,
    }
    return meta, in_parts


# ---------------------------------------------------------------------------
# Device kernel builder
# ---------------------------------------------------------------------------

def _build(meta):
    from contextlib import ExitStack

    import concourse.bass as bass  # noqa: F401
    import concourse.mybir as mybir
    import concourse.tile as tile
    from concourse import bacc

    f32 = mybir.dt.float32
    bf16 = mybir.dt.bfloat16
    i16 = mybir.dt.int16
    i32 = mybir.dt.int32
    n = meta["n"]
    t0 = meta["t0"]
    g0 = meta["g0"]
    nrank = meta["nrank"]
    S = meta["S"]
    colstart = meta["colstart"]
    tot_a, tot_b = meta["tot_cols"]
    uses = meta["uses"]
    u_tot = meta["u_tot"]
    b_base = meta["b_base"]
    queue_of = meta["queue_of"]
    n_dev = n + 2
    win_a = min(WINDOW_ROWS, n_dev)
    max_sa = int(max((S[g, 0] for g in range(g0)), default=1)) or 1
    max_sb = int(max((S[g, 1] for g in range(g0)), default=1)) or 1

    nc = bacc.Bacc("TRN2", target_bir_lowering=False, num_swdge_queues=N_QUEUES)
    x2_d = nc.dram_tensor("xrows", [n_dev, D], bf16, kind="ExternalInput")
    idxa_d = nc.dram_tensor("idxa", [P, max(tot_a * 8, 1)], i16, kind="ExternalInput")
    idxb_d = nc.dram_tensor("idxb", [P, max(tot_b * 8, 1)], i16, kind="ExternalInput")
    dstp_d = nc.dram_tensor("dstp", [P, u_tot], f32, kind="ExternalInput")
    recipe_d = nc.dram_tensor("recipe", [P, u_tot], f32, kind="ExternalInput")
    xt_d = nc.dram_tensor("xt", [P, nrank], bf16, kind="ExternalInput")
    wl_d = nc.dram_tensor("wl", [D, D], bf16, kind="ExternalInput")
    wr_d = nc.dram_tensor("wr", [D, D], bf16, kind="ExternalInput")
    b_d = nc.dram_tensor("bias", [D, 1], f32, kind="ExternalInput")
    out_d = nc.dram_tensor("outT", [P, nrank], f32, kind="ExternalOutput")

    with ExitStack() as ctx:
        tc = ctx.enter_context(tile.TileContext(nc))
        const = ctx.enter_context(tc.tile_pool(name="const", bufs=1))
        stga_pool = ctx.enter_context(tc.tile_pool(name="stga", bufs=3))
        stgb_pool = ctx.enter_context(tc.tile_pool(name="stgb", bufs=3))
        oh_pool = ctx.enter_context(tc.tile_pool(name="oh", bufs=6))
        mt_pool = ctx.enter_context(tc.tile_pool(name="mt", bufs=2))
        xt_pool = ctx.enter_context(tc.tile_pool(name="xtp", bufs=2))
        out_pool = ctx.enter_context(tc.tile_pool(name="outp", bufs=2))
        mt_psum = ctx.enter_context(tc.tile_pool(name="mtps", bufs=2, space="PSUM"))
        z_psum = ctx.enter_context(tc.tile_pool(name="zps", bufs=2, space="PSUM"))

        iota_i = const.tile([P, D], i32)
        nc.gpsimd.iota(iota_i[:], pattern=[[1, D]], base=0, channel_multiplier=0)
        iota_f = const.tile([P, D], bf16)
        nc.vector.tensor_copy(iota_f[:], iota_i[:])

        idxa_sb = const.tile([P, max(tot_a * 8, 1)], i16)
        nc.sync.dma_start(idxa_sb[:], idxa_d[:, :])
        idxb_sb = const.tile([P, max(tot_b * 8, 1)], i16)
        nc.sync.dma_start(idxb_sb[:], idxb_d[:, :])
        dstp_sb = const.tile([P, u_tot], f32)
        nc.sync.dma_start(dstp_sb[:], dstp_d[:, :])
        recipe_sb = const.tile([P, u_tot], f32)
        nc.sync.dma_start(recipe_sb[:], recipe_d[:, :])
        wl_sb = const.tile([D, D], bf16)
        nc.sync.dma_start(wl_sb[:], wl_d[:, :])
        wr_sb = const.tile([D, D], bf16)
        nc.sync.dma_start(wr_sb[:], wr_d[:, :])
        b_sb = const.tile([D, 1], f32)
        nc.sync.dma_start(b_sb[:], b_d[:, :])

        # zero the gather staging buffers once: trailing -1 indices leave
        # their SBUF rows unwritten, and stale SBUF garbage can decode to
        # NaN (NaN * 0 poisons the PSUM accumulation).
        for pool, sz in ((stga_pool, max_sa), (stgb_pool, max_sb)):
            for _ in range(3):
                t = pool.tile([P, sz * D], bf16, tag="stg_init")
                nc.vector.memset(t[:], 0.0)

        u_run = 0
        for g in range(g0):
            n_tiles = min(TPG, t0 - g * TPG)
            sa, sb = int(S[g, 0]), int(S[g, 1])
            stg = [None, None]
            if sa > 0:
                stg[0] = stga_pool.tile([P, max_sa * D], bf16, tag="stg_init",
                                        name=f"stga_{g}")
                nc.gpsimd.dma_gather(
                    out_ap=stg[0][:, :sa * D].rearrange("p (s e) -> p s e", e=D),
                    in_ap=x2_d[0:win_a, :],
                    idxs_ap=idxa_sb[:, colstart[g, 0] * 8:(colstart[g, 0] + sa) * 8],
                    num_idxs=sa * P,
                    num_idxs_reg=sa * P,
                    elem_size=D,
                    single_packet=False,
                    queue_num=int(queue_of[g, 0]),
                )
            if sb > 0:
                stg[1] = stgb_pool.tile([P, max_sb * D], bf16, tag="stg_init",
                                        name=f"stgb_{g}")
                nc.gpsimd.dma_gather(
                    out_ap=stg[1][:, :sb * D].rearrange("p (s e) -> p s e", e=D),
                    in_ap=x2_d[b_base:n_dev, :],
                    idxs_ap=idxb_sb[:, colstart[g, 1] * 8:(colstart[g, 1] + sb) * 8],
                    num_idxs=sb * P,
                    num_idxs_reg=sb * P,
                    elem_size=D,
                    single_packet=False,
                    queue_num=int(queue_of[g, 1]),
                )

            xt_sb = xt_pool.tile([P, TPG * D], bf16, tag="xt")
            nc.sync.dma_start(
                xt_sb[:, :n_tiles * D],
                xt_d[:, g * TPG * D:(g * TPG + n_tiles) * D],
            )

            mt_ps = mt_psum.tile([P, TPG * D], f32, space="PSUM")
            glist = uses[g]
            # first/last use index per tile for start/stop flags
            first_of = {}
            last_of = {}
            for pos, (wi, c, tl) in enumerate(glist):
                first_of.setdefault(tl, pos)
                last_of[tl] = pos
            for pos, (wi, c, tl) in enumerate(glist):
                oh = oh_pool.tile([P, D], bf16, tag="oh")
                uu = u_run + pos
                nc.vector.tensor_scalar(
                    out=oh[:],
                    in0=iota_f[:],
                    scalar1=dstp_sb[:, uu:uu + 1],
                    scalar2=recipe_sb[:, uu:uu + 1],
                    op0=mybir.AluOpType.is_equal,
                    op1=mybir.AluOpType.mult,
                )
                nc.tensor.matmul(
                    out=mt_ps[:, tl * D:(tl + 1) * D],
                    lhsT=stg[wi][:, c * D:(c + 1) * D],
                    rhs=oh[:],
                    start=(pos == first_of[tl]),
                    stop=(pos == last_of[tl]),
                )
            u_run += len(glist)

            mt_sb = mt_pool.tile([P, TPG * D], bf16, tag="mt")
            nc.scalar.copy(mt_sb[:, :n_tiles * D], mt_ps[:, :n_tiles * D])
            z_ps = z_psum.tile([P, TPG * D], f32, space="PSUM")
            nc.tensor.matmul(out=z_ps[:, :n_tiles * D], lhsT=wl_sb[:],
                             rhs=mt_sb[:, :n_tiles * D], start=True, stop=False)
            nc.tensor.matmul(out=z_ps[:, :n_tiles * D], lhsT=wr_sb[:],
                             rhs=xt_sb[:, :n_tiles * D], start=False, stop=True)
            o_sb = out_pool.tile([P, TPG * D], f32, tag="o")
            nc.scalar.activation(
                o_sb[:, :n_tiles * D], z_ps[:, :n_tiles * D],
                mybir.ActivationFunctionType.Relu, bias=b_sb[:, :1], scale=1.0,
            )
            nc.sync.dma_start(
                out_d[:, g * TPG * D:(g * TPG + n_tiles) * D],
                o_sb[:, :n_tiles * D],
            )

    nc.compile()
    return nc


# ---------------------------------------------------------------------------
# Top level
# ---------------------------------------------------------------------------

def _run(inputs, trace=False):
    from concourse import bass_utils

    x = np.ascontiguousarray(np.asarray(inputs["x"], dtype=np.float32))
    ei = np.asarray(inputs["edge_index"], dtype=np.int64)
    w_l = np.asarray(inputs["W_l"], dtype=np.float32)
    b_l = np.ascontiguousarray(np.asarray(inputs["b_l"], dtype=np.float32))
    w_r = np.asarray(inputs["W_r"], dtype=np.float32)
    src, dst = ei[0], ei[1]

    meta, in_parts = _prep(x, src, dst, N_CORES)
    nc = _build(meta)

    n = meta["n"]
    xrows = np.zeros((n + 2, D), dtype=np.float32)
    xrows[1:n + 1] = x
    xrows_bf = _bf16(xrows)
    wl_bf = _bf16(w_l)
    wr_bf = _bf16(w_r)
    b_col = np.ascontiguousarray(b_l.reshape(D, 1), dtype=np.float32)
    in_maps = []
    for m in range(N_CORES):
        part = in_parts[m]
        in_maps.append({
            "xrows": xrows_bf,
            "idxa": _pad_cols(part["idxa"]),
            "idxb": _pad_cols(part["idxb"]),
            "dstp": part["dstp"],
            "recipe": part["recipe"],
            "xt": part["xt"],
            "wl": wl_bf,
            "wr": wr_bf,
            "bias": b_col,
        })

    results = bass_utils.run_bass_kernel_spmd(
        nc, in_maps, core_ids=list(range(N_CORES)), trace=trace
    )

    npc = meta["npc"]
    out = np.empty((n, D), dtype=np.float32)
    for m in range(N_CORES):
        out_t = results.results[m]["outT"]  # [128, nrank] feature-major
        out[m * npc:(m + 1) * npc] = out_t[:, :npc].T
    return out, results


def _pad_cols(a):
    """int16 idx tables can be [128, 0]; the dram tensor is [128, >=1]."""
    if a.shape[1] == 0:
        return np.zeros((128, 1), dtype=np.int16)
    return a


def kernel(**inputs) -> np.ndarray:
    return _run(inputs)[0]


# revision 7
# speedup vs baseline: 1.4882x; 1.3316x over previous
"""GraphSAGE conv layer (PyG SAGEConv, aggr='mean') on 8 Trainium2 NeuronCores.

    out = relu(mean_j(x[src_j]) @ W_l + b_l + x @ W_r)

Sharding: edges are partitioned by destination node across the 8 cores (6250
destination nodes per core); the small 128x128 weights are replicated; each
core keeps a full copy of x in its DRAM so the per-edge source-feature gather
stays local (full-input replication instead of a halo exchange).

Per-core device pipeline (bf16 data, f32 accumulation):
  - Edges are bucketed host-side by (512-node destination group, source-index
    window) and fetched edge-major with bulk `dma_gather` ops. Gather calls
    are striped across all 4 SWDGE queues so all four Q7 core pairs generate
    DMA descriptors concurrently (descriptor generation is the dominant cost).
  - Per-core padding slack in the shared-size index tables is -1, which the
    Q7 gather kernel skips entirely (no descriptors, no DMA).
  - For each 128-edge column the DVE builds a scaled one-hot selector in bf16
    (is_equal against an iota, times 1/deg of the destination) and the PE
    contracts messages^T @ onehot into PSUM, accumulating the feature-major
    per-node mean directly (segment mean == one matmul chain per node tile).
  - Weight-stationary bf16 matmuls add W_l.T @ meanT + W_r.T @ xT, ACT fuses
    bias + ReLU, and the result is stored feature-major; the host transposes
    while assembling the full output.
"""

import math

import numpy as np

N_CORES = 8
D = 128
P = 128
TPG = 4           # node tiles (of 128 nodes) per PSUM group -> 512 wide
WINDOW_ROWS = 32768   # dma_gather int16 index window (rows)
N_QUEUES = 4          # SWDGE queues (Q7 core pairs) to stripe gathers across


def _bf16(a):
    import ml_dtypes
    return np.ascontiguousarray(np.asarray(a).astype(ml_dtypes.bfloat16))


# ---------------------------------------------------------------------------
# Host-side sharding / table prep
# ---------------------------------------------------------------------------

def _prep(x, src, dst, n_cores):
    n, d = x.shape
    assert d == D
    npc = n // n_cores
    assert npc * n_cores == n
    t0 = math.ceil(npc / P)          # node tiles per core
    g0 = math.ceil(t0 / TPG)         # groups per core
    nrank = t0 * P

    # source windows over the device x copy: row 0 and row n+1 are zeros,
    # x row i lives at device row i+1 (50002 rows total).
    n_dev = n + 2
    assert 2 * WINDOW_ROWS >= n_dev, "two windows must cover all of x"
    a_max_src = min(WINDOW_ROWS - 1, n_dev - 1) - 1   # src s -> row s+1
    b_base = max(0, n_dev - WINDOW_ROWS)
    b_pad = min(n_dev - 1, WINDOW_ROWS - 1)           # window-local zero row

    per_core = []
    cnt = np.zeros((n_cores, g0, 2), dtype=np.int64)
    # per (core, g, w, t_loc): first/last edge position inside the call
    starts3 = np.full((n_cores, g0, 2, TPG), -1, dtype=np.int64)
    ends3 = np.full((n_cores, g0, 2, TPG), -1, dtype=np.int64)

    for m in range(n_cores):
        sel = (dst >= m * npc) & (dst < (m + 1) * npc)
        s = src[sel]
        dl = dst[sel] - m * npc
        deg = np.bincount(dl, minlength=npc)
        recip_node = (1.0 / np.maximum(deg, 1)).astype(np.float32)
        tile = dl // P
        grp = tile // TPG
        w = (s > a_max_src).astype(np.int64)
        order = np.lexsort((tile, w, grp))
        s, dl, tile, grp, w = s[order], dl[order], tile[order], grp[order], w[order]
        t_loc = tile - grp * TPG

        for g in range(g0):
            for wi in range(2):
                selgw = (grp == g) & (w == wi)
                cnt[m, g, wi] = selgw.sum()
                if cnt[m, g, wi] == 0:
                    continue
                base = np.nonzero(selgw)[0][0]
                for tl in range(TPG):
                    st = (grp == g) & (w == wi) & (t_loc == tl)
                    c = st.sum()
                    if c == 0:
                        continue
                    first = np.nonzero(st)[0][0] - base
                    starts3[m, g, wi, tl] = first
                    ends3[m, g, wi, tl] = first + c
        per_core.append((s, dl, grp, w, t_loc, recip_node))

    # shared call sizes (in 128-edge columns)
    S = np.ceil(cnt / P).astype(np.int64).max(axis=0)     # [g0, 2]
    colstart = np.zeros((g0, 2), dtype=np.int64)          # per-window cumulative
    acc = [0, 0]
    for g in range(g0):
        for wi in range(2):
            colstart[g, wi] = acc[wi]
            acc[wi] += S[g, wi]
    tot_cols = (acc[0], acc[1])

    # stripe gather calls across SWDGE queues, greedy-balanced by size
    qloads = [0] * N_QUEUES
    queue_of = np.zeros((g0, 2), dtype=np.int64)
    for g in range(g0):
        for wi in range(2):
            if S[g, wi] == 0:
                continue
            q = min(range(N_QUEUES), key=lambda i: qloads[i])
            queue_of[g, wi] = q
            qloads[q] += int(S[g, wi])

    # shared use lists: per (g, w, t_loc) the union (over cores) column range
    uses = [[] for _ in range(g0)]    # per group: list of (w, c, t_loc)
    use_c0 = np.full((g0, 2, TPG), -1, dtype=np.int64)
    use_u0 = np.full((g0, 2, TPG), -1, dtype=np.int64)
    u_tot = 0
    for g in range(g0):
        n_tiles = min(TPG, t0 - g * TPG)
        for tl in range(n_tiles):
            tile_uses = []
            for wi in range(2):
                stm = starts3[:, g, wi, tl]
                enm = ends3[:, g, wi, tl]
                anyc = stm >= 0
                if not anyc.any():
                    continue
                c_lo = int((stm[anyc] // P).min())
                c_hi = int(((enm[anyc] - 1) // P).max())
                use_c0[g, wi, tl] = c_lo
                use_u0[g, wi, tl] = u_tot + len(tile_uses)
                for c in range(c_lo, c_hi + 1):
                    tile_uses.append((wi, c, tl))
            if not tile_uses:
                # keep the PSUM slice defined: one all-masked use
                wi = 0 if S[g, 0] > 0 else 1
                assert S[g, wi] > 0, f"group {g} has no gather columns at all"
                tile_uses.append((wi, 0, tl))
            uses[g].extend(tile_uses)
    u_tot = sum(len(u) for u in uses)

    # global u index per (g, position-in-group-list)
    u_base = np.zeros(g0, dtype=np.int64)
    accu = 0
    for g in range(g0):
        u_base[g] = accu
        accu += len(uses[g])

    # map (g, w, t_loc, c) -> global u (uses within a tile are consecutive cols)
    u_lookup = {}
    for g in range(g0):
        for pos, (wi, c, tl) in enumerate(uses[g]):
            u_lookup[(g, wi, tl, c)] = u_base[g] + pos

    in_parts = []
    for m in range(n_cores):
        s, dl, grp, w, t_loc, recip_node = per_core[m]
        idx_t = [
            np.zeros(int(tot_cols[0]) * P, dtype=np.int16),
            np.full(int(tot_cols[1]) * P, b_pad, dtype=np.int16),
        ]
        dstp = np.full((P, u_tot), -1.0, dtype=np.float32)
        recipe = np.zeros((P, u_tot), dtype=np.float32)

        # per-edge position j inside its (g, w) call
        call_of_edge = grp * 2 + w
        call_sizes = np.bincount(call_of_edge, minlength=g0 * 2)
        call_first = np.concatenate([[0], np.cumsum(call_sizes)])[:-1]
        j = np.arange(len(s)) - call_first[call_of_edge]
        col = j // P
        p = j % P

        # gather index values
        lin = (colstart[grp, w] * P + j).astype(np.int64)
        val_a = (s + 1).astype(np.int64)
        val_b = (s + 1 - b_base).astype(np.int64)
        isa = w == 0
        idx_t[0][lin[isa]] = val_a[isa]
        idx_t[1][lin[~isa]] = val_b[~isa]

        # dstloc / recip tables per use
        u_edge = np.empty(len(s), dtype=np.int64)
        for g in range(g0):
            for wi in range(2):
                for tl in range(TPG):
                    selgwt = (grp == g) & (w == wi) & (t_loc == tl)
                    if not selgwt.any():
                        continue
                    u0 = u_lookup[(g, wi, tl, int(use_c0[g, wi, tl]))]
                    c0 = use_c0[g, wi, tl]
                    u_edge[selgwt] = u0 + (col[selgwt] - c0)
        dstp[p, u_edge] = (dl - (grp * TPG + t_loc) * P).astype(np.float32)
        recipe[p, u_edge] = recip_node[dl]

        # wrap idx tables to [128, n/16]: linear idx i lives at
        # [i % 16, i // 16], replicated 8x down the partition dim (one copy
        # per Q7 core pair).
        def wrap(a):
            if len(a) == 0:
                return np.zeros((128, 0), dtype=np.int16)
            w16 = a.reshape(-1, 16).T
            return np.ascontiguousarray(np.tile(w16, (8, 1)))

        xt = np.zeros((P, nrank), dtype=np.float32)
        xt[:, :npc] = x[m * npc:(m + 1) * npc].T

        in_parts.append({
            "idxa": wrap(idx_t[0]),
            "idxb": wrap(idx_t[1]),
            "dstp": dstp,
            "recipe": recipe,
            "xt": _bf16(xt),
        })

    meta = {
        "n": n, "npc": npc, "t0": t0, "g0": g0, "nrank": nrank,
        "S": S, "colstart": colstart, "tot_cols": tot_cols,
        "uses": uses, "u_tot": u_tot, "b_base": b_base,
        "queue_of": queue_of,
    }
    return meta, in_parts


# ---------------------------------------------------------------------------
# Device kernel builder
# ---------------------------------------------------------------------------

def _build(meta):
    from contextlib import ExitStack

    import concourse.bass as bass  # noqa: F401
    import concourse.mybir as mybir
    import concourse.tile as tile
    from concourse import bacc

    f32 = mybir.dt.float32
    bf16 = mybir.dt.bfloat16
    i16 = mybir.dt.int16
    i32 = mybir.dt.int32
    n = meta["n"]
    t0 = meta["t0"]
    g0 = meta["g0"]
    nrank = meta["nrank"]
    S = meta["S"]
    colstart = meta["colstart"]
    tot_a, tot_b = meta["tot_cols"]
    uses = meta["uses"]
    u_tot = meta["u_tot"]
    b_base = meta["b_base"]
    queue_of = meta["queue_of"]
    n_dev = n + 2
    win_a = min(WINDOW_ROWS, n_dev)
    max_sa = int(max((S[g, 0] for g in range(g0)), default=1)) or 1
    max_sb = int(max((S[g, 1] for g in range(g0)), default=1)) or 1

    nc = bacc.Bacc("TRN2", target_bir_lowering=False, num_swdge_queues=N_QUEUES)
    x2_d = nc.dram_tensor("xrows", [n_dev, D], bf16, kind="ExternalInput")
    idxa_d = nc.dram_tensor("idxa", [P, max(tot_a * 8, 1)], i16, kind="ExternalInput")
    idxb_d = nc.dram_tensor("idxb", [P, max(tot_b * 8, 1)], i16, kind="ExternalInput")
    dstp_d = nc.dram_tensor("dstp", [P, u_tot], f32, kind="ExternalInput")
    recipe_d = nc.dram_tensor("recipe", [P, u_tot], f32, kind="ExternalInput")
    xt_d = nc.dram_tensor("xt", [P, nrank], bf16, kind="ExternalInput")
    wl_d = nc.dram_tensor("wl", [D, D], bf16, kind="ExternalInput")
    wr_d = nc.dram_tensor("wr", [D, D], bf16, kind="ExternalInput")
    b_d = nc.dram_tensor("bias", [D, 1], f32, kind="ExternalInput")
    out_d = nc.dram_tensor("outT", [P, nrank], f32, kind="ExternalOutput")

    with ExitStack() as ctx:
        tc = ctx.enter_context(tile.TileContext(nc))
        const = ctx.enter_context(tc.tile_pool(name="const", bufs=1))
        stga_pool = ctx.enter_context(tc.tile_pool(name="stga", bufs=3))
        stgb_pool = ctx.enter_context(tc.tile_pool(name="stgb", bufs=3))
        oh_pool = ctx.enter_context(tc.tile_pool(name="oh", bufs=6))
        mt_pool = ctx.enter_context(tc.tile_pool(name="mt", bufs=2))
        xt_pool = ctx.enter_context(tc.tile_pool(name="xtp", bufs=2))
        out_pool = ctx.enter_context(tc.tile_pool(name="outp", bufs=2))
        mt_psum = ctx.enter_context(tc.tile_pool(name="mtps", bufs=2, space="PSUM"))
        z_psum = ctx.enter_context(tc.tile_pool(name="zps", bufs=2, space="PSUM"))

        iota_i = const.tile([P, D], i32)
        nc.gpsimd.iota(iota_i[:], pattern=[[1, D]], base=0, channel_multiplier=0)
        iota_f = const.tile([P, D], bf16)
        nc.vector.tensor_copy(iota_f[:], iota_i[:])

        idxa_sb = const.tile([P, max(tot_a * 8, 1)], i16)
        nc.sync.dma_start(idxa_sb[:], idxa_d[:, :])
        idxb_sb = const.tile([P, max(tot_b * 8, 1)], i16)
        nc.sync.dma_start(idxb_sb[:], idxb_d[:, :])
        dstp_sb = const.tile([P, u_tot], f32)
        nc.sync.dma_start(dstp_sb[:], dstp_d[:, :])
        recipe_sb = const.tile([P, u_tot], f32)
        nc.sync.dma_start(recipe_sb[:], recipe_d[:, :])
        wl_sb = const.tile([D, D], bf16)
        nc.sync.dma_start(wl_sb[:], wl_d[:, :])
        wr_sb = const.tile([D, D], bf16)
        nc.sync.dma_start(wr_sb[:], wr_d[:, :])
        b_sb = const.tile([D, 1], f32)
        nc.sync.dma_start(b_sb[:], b_d[:, :])

        # zero the gather staging buffers once: trailing -1 indices leave
        # their SBUF rows unwritten, and stale SBUF garbage can decode to
        # NaN (NaN * 0 poisons the PSUM accumulation).
        for pool, sz in ((stga_pool, max_sa), (stgb_pool, max_sb)):
            for _ in range(3):
                t = pool.tile([P, sz * D], bf16, tag="stg_init")
                nc.vector.memset(t[:], 0.0)

        u_run = 0
        for g in range(g0):
            n_tiles = min(TPG, t0 - g * TPG)
            sa, sb = int(S[g, 0]), int(S[g, 1])
            stg = [None, None]
            if sa > 0:
                stg[0] = stga_pool.tile([P, max_sa * D], bf16, tag="stg_init",
                                        name=f"stga_{g}")
                nc.gpsimd.dma_gather(
                    out_ap=stg[0][:, :sa * D].rearrange("p (s e) -> p s e", e=D),
                    in_ap=x2_d[0:win_a, :],
                    idxs_ap=idxa_sb[:, colstart[g, 0] * 8:(colstart[g, 0] + sa) * 8],
                    num_idxs=sa * P,
                    num_idxs_reg=sa * P,
                    elem_size=D,
                    single_packet=False,
                    queue_num=int(queue_of[g, 0]),
                )
            if sb > 0:
                stg[1] = stgb_pool.tile([P, max_sb * D], bf16, tag="stg_init",
                                        name=f"stgb_{g}")
                nc.gpsimd.dma_gather(
                    out_ap=stg[1][:, :sb * D].rearrange("p (s e) -> p s e", e=D),
                    in_ap=x2_d[b_base:n_dev, :],
                    idxs_ap=idxb_sb[:, colstart[g, 1] * 8:(colstart[g, 1] + sb) * 8],
                    num_idxs=sb * P,
                    num_idxs_reg=sb * P,
                    elem_size=D,
                    single_packet=False,
                    queue_num=int(queue_of[g, 1]),
                )

            xt_sb = xt_pool.tile([P, TPG * D], bf16, tag="xt")
            nc.sync.dma_start(
                xt_sb[:, :n_tiles * D],
                xt_d[:, g * TPG * D:(g * TPG + n_tiles) * D],
            )

            mt_ps = mt_psum.tile([P, TPG * D], f32, space="PSUM")
            glist = uses[g]
            # first/last use index per tile for start/stop flags
            first_of = {}
            last_of = {}
            for pos, (wi, c, tl) in enumerate(glist):
                first_of.setdefault(tl, pos)
                last_of[tl] = pos
            for pos, (wi, c, tl) in enumerate(glist):
                oh = oh_pool.tile([P, D], bf16, tag="oh")
                uu = u_run + pos
                nc.vector.tensor_scalar(
                    out=oh[:],
                    in0=iota_f[:],
                    scalar1=dstp_sb[:, uu:uu + 1],
                    scalar2=recipe_sb[:, uu:uu + 1],
                    op0=mybir.AluOpType.is_equal,
                    op1=mybir.AluOpType.mult,
                )
                nc.tensor.matmul(
                    out=mt_ps[:, tl * D:(tl + 1) * D],
                    lhsT=stg[wi][:, c * D:(c + 1) * D],
                    rhs=oh[:],
                    start=(pos == first_of[tl]),
                    stop=(pos == last_of[tl]),
                )
            u_run += len(glist)

            mt_sb = mt_pool.tile([P, TPG * D], bf16, tag="mt")
            nc.scalar.copy(mt_sb[:, :n_tiles * D], mt_ps[:, :n_tiles * D])
            z_ps = z_psum.tile([P, TPG * D], f32, space="PSUM")
            nc.tensor.matmul(out=z_ps[:, :n_tiles * D], lhsT=wl_sb[:],
                             rhs=mt_sb[:, :n_tiles * D], start=True, stop=False)
            nc.tensor.matmul(out=z_ps[:, :n_tiles * D], lhsT=wr_sb[:],
                             rhs=xt_sb[:, :n_tiles * D], start=False, stop=True)
            o_sb = out_pool.tile([P, TPG * D], f32, tag="o")
            nc.scalar.activation(
                o_sb[:, :n_tiles * D], z_ps[:, :n_tiles * D],
                mybir.ActivationFunctionType.Relu, bias=b_sb[:, :1], scale=1.0,
            )
            nc.sync.dma_start(
                out_d[:, g * TPG * D:(g * TPG + n_tiles) * D],
                o_sb[:, :n_tiles * D],
            )

    nc.compile()
    return nc


# ---------------------------------------------------------------------------
# Top level
# ---------------------------------------------------------------------------

def _run(inputs, trace=False):
    from concourse import bass_utils

    x = np.ascontiguousarray(np.asarray(inputs["x"], dtype=np.float32))
    ei = np.asarray(inputs["edge_index"], dtype=np.int64)
    w_l = np.asarray(inputs["W_l"], dtype=np.float32)
    b_l = np.ascontiguousarray(np.asarray(inputs["b_l"], dtype=np.float32))
    w_r = np.asarray(inputs["W_r"], dtype=np.float32)
    src, dst = ei[0], ei[1]

    meta, in_parts = _prep(x, src, dst, N_CORES)
    nc = _build(meta)

    n = meta["n"]
    xrows = np.zeros((n + 2, D), dtype=np.float32)
    xrows[1:n + 1] = x
    xrows_bf = _bf16(xrows)
    wl_bf = _bf16(w_l)
    wr_bf = _bf16(w_r)
    b_col = np.ascontiguousarray(b_l.reshape(D, 1), dtype=np.float32)
    in_maps = []
    for m in range(N_CORES):
        part = in_parts[m]
        in_maps.append({
            "xrows": xrows_bf,
            "idxa": _pad_cols(part["idxa"]),
            "idxb": _pad_cols(part["idxb"]),
            "dstp": part["dstp"],
            "recipe": part["recipe"],
            "xt": part["xt"],
            "wl": wl_bf,
            "wr": wr_bf,
            "bias": b_col,
        })

    results = bass_utils.run_bass_kernel_spmd(
        nc, in_maps, core_ids=list(range(N_CORES)), trace=trace
    )

    npc = meta["npc"]
    out = np.empty((n, D), dtype=np.float32)
    for m in range(N_CORES):
        out_t = results.results[m]["outT"]  # [128, nrank] feature-major
        out[m * npc:(m + 1) * npc] = out_t[:, :npc].T
    return out, results


def _pad_cols(a):
    """int16 idx tables can be [128, 0]; the dram tensor is [128, >=1]."""
    if a.shape[1] == 0:
        return np.zeros((128, 1), dtype=np.int16)
    return a


def kernel(**inputs) -> np.ndarray:
    return _run(inputs)[0]


# revision 8
# speedup vs baseline: 1.5563x; 1.0458x over previous
"""GraphSAGE conv layer (PyG SAGEConv, aggr='mean') on 8 Trainium2 NeuronCores.

    out = relu(mean_j(x[src_j]) @ W_l + b_l + x @ W_r)

Sharding: edges are partitioned by destination node across the 8 cores (6250
destination nodes per core); the small 128x128 weights are replicated; each
core keeps a full copy of x in its DRAM so the per-edge source-feature gather
stays local (full-input replication instead of a halo exchange).

Per-core device pipeline (bf16 data, f32 accumulation):
  - Edges are bucketed host-side by (512-node destination group, source-index
    window) and fetched edge-major with bulk `dma_gather` ops. Gather calls
    are striped across all 4 SWDGE queues so all four Q7 core pairs generate
    DMA descriptors concurrently (descriptor generation is the dominant cost).
  - Per-core padding slack in the shared-size index tables is -1, which the
    Q7 gather kernel skips entirely (no descriptors, no DMA).
  - For each 128-edge column the DVE builds a scaled one-hot selector in bf16
    (is_equal against an iota, times 1/deg of the destination) and the PE
    contracts messages^T @ onehot into PSUM, accumulating the feature-major
    per-node mean directly (segment mean == one matmul chain per node tile).
  - Weight-stationary bf16 matmuls add W_l.T @ meanT + W_r.T @ xT, ACT fuses
    bias + ReLU, and the result is stored feature-major; the host transposes
    while assembling the full output.
"""

import math

import numpy as np

N_CORES = 8
D = 128
P = 128
TPG = 4           # node tiles (of 128 nodes) per PSUM group -> 512 wide
WINDOW_ROWS = 32768   # dma_gather int16 index window (rows)
N_QUEUES = 4          # SWDGE queues (Q7 core pairs) to stripe gathers across


def _bf16(a):
    import ml_dtypes
    return np.ascontiguousarray(np.asarray(a).astype(ml_dtypes.bfloat16))


# ---------------------------------------------------------------------------
# Host-side sharding / table prep
# ---------------------------------------------------------------------------

def _prep(x, src, dst, n_cores):
    n, d = x.shape
    assert d == D
    npc = n // n_cores
    assert npc * n_cores == n
    t0 = math.ceil(npc / P)          # node tiles per core
    g0 = math.ceil(t0 / TPG)         # groups per core
    nrank = t0 * P

    # source windows over the device x copy: row 0 and row n+1 are zeros,
    # x row i lives at device row i+1 (50002 rows total).
    n_dev = n + 2
    assert 2 * WINDOW_ROWS >= n_dev, "two windows must cover all of x"
    a_max_src = min(WINDOW_ROWS - 1, n_dev - 1) - 1   # src s -> row s+1
    b_base = max(0, n_dev - WINDOW_ROWS)
    b_pad = min(n_dev - 1, WINDOW_ROWS - 1)           # window-local zero row

    per_core = []
    cnt = np.zeros((n_cores, g0, 2), dtype=np.int64)
    # per (core, g, w, t_loc): first/last edge position inside the call
    starts3 = np.full((n_cores, g0, 2, TPG), -1, dtype=np.int64)
    ends3 = np.full((n_cores, g0, 2, TPG), -1, dtype=np.int64)

    for m in range(n_cores):
        sel = (dst >= m * npc) & (dst < (m + 1) * npc)
        s = src[sel]
        dl = dst[sel] - m * npc
        deg = np.bincount(dl, minlength=npc)
        recip_node = (1.0 / np.maximum(deg, 1)).astype(np.float32)
        tile = dl // P
        grp = tile // TPG
        w = (s > a_max_src).astype(np.int64)
        order = np.lexsort((tile, w, grp))
        s, dl, tile, grp, w = s[order], dl[order], tile[order], grp[order], w[order]
        t_loc = tile - grp * TPG

        for g in range(g0):
            for wi in range(2):
                selgw = (grp == g) & (w == wi)
                cnt[m, g, wi] = selgw.sum()
                if cnt[m, g, wi] == 0:
                    continue
                base = np.nonzero(selgw)[0][0]
                for tl in range(TPG):
                    st = (grp == g) & (w == wi) & (t_loc == tl)
                    c = st.sum()
                    if c == 0:
                        continue
                    first = np.nonzero(st)[0][0] - base
                    starts3[m, g, wi, tl] = first
                    ends3[m, g, wi, tl] = first + c
        per_core.append((s, dl, grp, w, t_loc, recip_node))

    # shared call sizes (in 128-edge columns)
    S = np.ceil(cnt / P).astype(np.int64).max(axis=0)     # [g0, 2]
    colstart = np.zeros((g0, 2), dtype=np.int64)          # per-window cumulative
    acc = [0, 0]
    for g in range(g0):
        for wi in range(2):
            colstart[g, wi] = acc[wi]
            acc[wi] += S[g, wi]
    tot_cols = (acc[0], acc[1])

    # stripe gather calls across SWDGE queues, greedy-balanced by size
    qloads = [0] * N_QUEUES
    queue_of = np.zeros((g0, 2), dtype=np.int64)
    for g in range(g0):
        for wi in range(2):
            if S[g, wi] == 0:
                continue
            q = min(range(N_QUEUES), key=lambda i: qloads[i])
            queue_of[g, wi] = q
            qloads[q] += int(S[g, wi])

    # shared use lists: per (g, w, t_loc) the union (over cores) column range
    uses = [[] for _ in range(g0)]    # per group: list of (w, c, t_loc)
    use_c0 = np.full((g0, 2, TPG), -1, dtype=np.int64)
    use_u0 = np.full((g0, 2, TPG), -1, dtype=np.int64)
    u_tot = 0
    for g in range(g0):
        n_tiles = min(TPG, t0 - g * TPG)
        for tl in range(n_tiles):
            tile_uses = []
            for wi in range(2):
                stm = starts3[:, g, wi, tl]
                enm = ends3[:, g, wi, tl]
                anyc = stm >= 0
                if not anyc.any():
                    continue
                c_lo = int((stm[anyc] // P).min())
                c_hi = int(((enm[anyc] - 1) // P).max())
                use_c0[g, wi, tl] = c_lo
                use_u0[g, wi, tl] = u_tot + len(tile_uses)
                for c in range(c_lo, c_hi + 1):
                    tile_uses.append((wi, c, tl))
            if not tile_uses:
                # keep the PSUM slice defined: one all-masked use
                wi = 0 if S[g, 0] > 0 else 1
                assert S[g, wi] > 0, f"group {g} has no gather columns at all"
                tile_uses.append((wi, 0, tl))
            uses[g].extend(tile_uses)
    u_tot = sum(len(u) for u in uses)

    # global u index per (g, position-in-group-list)
    u_base = np.zeros(g0, dtype=np.int64)
    accu = 0
    for g in range(g0):
        u_base[g] = accu
        accu += len(uses[g])

    # map (g, w, t_loc, c) -> global u (uses within a tile are consecutive cols)
    u_lookup = {}
    for g in range(g0):
        for pos, (wi, c, tl) in enumerate(uses[g]):
            u_lookup[(g, wi, tl, c)] = u_base[g] + pos

    in_parts = []
    for m in range(n_cores):
        s, dl, grp, w, t_loc, recip_node = per_core[m]
        idx_t = [
            np.zeros(int(tot_cols[0]) * P, dtype=np.int16),
            np.full(int(tot_cols[1]) * P, b_pad, dtype=np.int16),
        ]
        dstp = np.full((P, u_tot), -1.0, dtype=np.float32)
        recipe = np.zeros((P, u_tot), dtype=np.float32)

        # per-edge position j inside its (g, w) call
        call_of_edge = grp * 2 + w
        call_sizes = np.bincount(call_of_edge, minlength=g0 * 2)
        call_first = np.concatenate([[0], np.cumsum(call_sizes)])[:-1]
        j = np.arange(len(s)) - call_first[call_of_edge]
        col = j // P
        p = j % P

        # gather index values
        lin = (colstart[grp, w] * P + j).astype(np.int64)
        val_a = (s + 1).astype(np.int64)
        val_b = (s + 1 - b_base).astype(np.int64)
        isa = w == 0
        idx_t[0][lin[isa]] = val_a[isa]
        idx_t[1][lin[~isa]] = val_b[~isa]

        # dstloc / recip tables per use
        u_edge = np.empty(len(s), dtype=np.int64)
        for g in range(g0):
            for wi in range(2):
                for tl in range(TPG):
                    selgwt = (grp == g) & (w == wi) & (t_loc == tl)
                    if not selgwt.any():
                        continue
                    u0 = u_lookup[(g, wi, tl, int(use_c0[g, wi, tl]))]
                    c0 = use_c0[g, wi, tl]
                    u_edge[selgwt] = u0 + (col[selgwt] - c0)
        dstp[p, u_edge] = (dl - (grp * TPG + t_loc) * P).astype(np.float32)
        recipe[p, u_edge] = recip_node[dl]

        # wrap idx tables to [128, n/16]: linear idx i lives at
        # [i % 16, i // 16], replicated 8x down the partition dim (one copy
        # per Q7 core pair).
        def wrap(a):
            if len(a) == 0:
                return np.zeros((128, 0), dtype=np.int16)
            w16 = a.reshape(-1, 16).T
            return np.ascontiguousarray(np.tile(w16, (8, 1)))

        xt = np.zeros((P, nrank), dtype=np.float32)
        xt[:, :npc] = x[m * npc:(m + 1) * npc].T

        in_parts.append({
            "idxa": wrap(idx_t[0]),
            "idxb": wrap(idx_t[1]),
            "dstp": dstp,
            "recipe": recipe,
            "xt": _bf16(xt),
        })

    meta = {
        "n": n, "npc": npc, "t0": t0, "g0": g0, "nrank": nrank,
        "S": S, "colstart": colstart, "tot_cols": tot_cols,
        "uses": uses, "u_tot": u_tot, "b_base": b_base,
        "queue_of": queue_of,
    }
    return meta, in_parts


# ---------------------------------------------------------------------------
# Device kernel builder
# ---------------------------------------------------------------------------

def _build(meta):
    from contextlib import ExitStack

    import concourse.bass as bass  # noqa: F401
    import concourse.mybir as mybir
    import concourse.tile as tile
    from concourse import bacc

    f32 = mybir.dt.float32
    bf16 = mybir.dt.bfloat16
    i16 = mybir.dt.int16
    i32 = mybir.dt.int32
    n = meta["n"]
    t0 = meta["t0"]
    g0 = meta["g0"]
    nrank = meta["nrank"]
    S = meta["S"]
    colstart = meta["colstart"]
    tot_a, tot_b = meta["tot_cols"]
    uses = meta["uses"]
    u_tot = meta["u_tot"]
    b_base = meta["b_base"]
    queue_of = meta["queue_of"]
    n_dev = n + 2
    win_a = min(WINDOW_ROWS, n_dev)
    max_sa = int(max((S[g, 0] for g in range(g0)), default=1)) or 1
    max_sb = int(max((S[g, 1] for g in range(g0)), default=1)) or 1

    nc = bacc.Bacc("TRN2", target_bir_lowering=False, num_swdge_queues=N_QUEUES)
    x2_d = nc.dram_tensor("xrows", [n_dev, D], bf16, kind="ExternalInput")
    idxa_d = nc.dram_tensor("idxa", [P, max(tot_a * 8, 1)], i16, kind="ExternalInput")
    idxb_d = nc.dram_tensor("idxb", [P, max(tot_b * 8, 1)], i16, kind="ExternalInput")
    dstp_d = nc.dram_tensor("dstp", [P, u_tot], f32, kind="ExternalInput")
    recipe_d = nc.dram_tensor("recipe", [P, u_tot], f32, kind="ExternalInput")
    xt_d = nc.dram_tensor("xt", [P, nrank], bf16, kind="ExternalInput")
    wl_d = nc.dram_tensor("wl", [D, D], bf16, kind="ExternalInput")
    wr_d = nc.dram_tensor("wr", [D, D], bf16, kind="ExternalInput")
    b_d = nc.dram_tensor("bias", [D, 1], f32, kind="ExternalInput")
    out_d = nc.dram_tensor("outT", [P, nrank], f32, kind="ExternalOutput")

    with ExitStack() as ctx:
        tc = ctx.enter_context(tile.TileContext(nc))
        const = ctx.enter_context(tc.tile_pool(name="const", bufs=1))
        stga_pool = ctx.enter_context(tc.tile_pool(name="stga", bufs=6))
        stgb_pool = ctx.enter_context(tc.tile_pool(name="stgb", bufs=6))
        oh_pool = ctx.enter_context(tc.tile_pool(name="oh", bufs=24))
        mt_pool = ctx.enter_context(tc.tile_pool(name="mt", bufs=3))
        xt_pool = ctx.enter_context(tc.tile_pool(name="xtp", bufs=3))
        out_pool = ctx.enter_context(tc.tile_pool(name="outp", bufs=3))
        mt_psum = ctx.enter_context(tc.tile_pool(name="mtps", bufs=3, space="PSUM"))
        z_psum = ctx.enter_context(tc.tile_pool(name="zps", bufs=3, space="PSUM"))

        iota_i = const.tile([P, D], i32)
        nc.gpsimd.iota(iota_i[:], pattern=[[1, D]], base=0, channel_multiplier=0)
        iota_f = const.tile([P, D], bf16)
        nc.vector.tensor_copy(iota_f[:], iota_i[:])

        idxa_sb = const.tile([P, max(tot_a * 8, 1)], i16)
        nc.sync.dma_start(idxa_sb[:], idxa_d[:, :])
        idxb_sb = const.tile([P, max(tot_b * 8, 1)], i16)
        nc.sync.dma_start(idxb_sb[:], idxb_d[:, :])
        dstp_sb = const.tile([P, u_tot], f32)
        nc.sync.dma_start(dstp_sb[:], dstp_d[:, :])
        recipe_sb = const.tile([P, u_tot], f32)
        nc.sync.dma_start(recipe_sb[:], recipe_d[:, :])
        wl_sb = const.tile([D, D], bf16)
        nc.sync.dma_start(wl_sb[:], wl_d[:, :])
        wr_sb = const.tile([D, D], bf16)
        nc.sync.dma_start(wr_sb[:], wr_d[:, :])
        b_sb = const.tile([D, 1], f32)
        nc.sync.dma_start(b_sb[:], b_d[:, :])

        # zero the gather staging buffers once: trailing -1 indices leave
        # their SBUF rows unwritten, and stale SBUF garbage can decode to
        # NaN (NaN * 0 poisons the PSUM accumulation).
        for pool, sz in ((stga_pool, max_sa), (stgb_pool, max_sb)):
            for _ in range(6):
                t = pool.tile([P, sz * D], bf16, tag="stg_init")
                nc.vector.memset(t[:], 0.0)

        u_run = 0
        for g in range(g0):
            n_tiles = min(TPG, t0 - g * TPG)
            sa, sb = int(S[g, 0]), int(S[g, 1])
            stg = [None, None]
            if sa > 0:
                stg[0] = stga_pool.tile([P, max_sa * D], bf16, tag="stg_init",
                                        name=f"stga_{g}")
                nc.gpsimd.dma_gather(
                    out_ap=stg[0][:, :sa * D].rearrange("p (s e) -> p s e", e=D),
                    in_ap=x2_d[0:win_a, :],
                    idxs_ap=idxa_sb[:, colstart[g, 0] * 8:(colstart[g, 0] + sa) * 8],
                    num_idxs=sa * P,
                    num_idxs_reg=sa * P,
                    elem_size=D,
                    single_packet=False,
                    queue_num=int(queue_of[g, 0]),
                )
            if sb > 0:
                stg[1] = stgb_pool.tile([P, max_sb * D], bf16, tag="stg_init",
                                        name=f"stgb_{g}")
                nc.gpsimd.dma_gather(
                    out_ap=stg[1][:, :sb * D].rearrange("p (s e) -> p s e", e=D),
                    in_ap=x2_d[b_base:n_dev, :],
                    idxs_ap=idxb_sb[:, colstart[g, 1] * 8:(colstart[g, 1] + sb) * 8],
                    num_idxs=sb * P,
                    num_idxs_reg=sb * P,
                    elem_size=D,
                    single_packet=False,
                    queue_num=int(queue_of[g, 1]),
                )

            xt_sb = xt_pool.tile([P, TPG * D], bf16, tag="xt")
            nc.sync.dma_start(
                xt_sb[:, :n_tiles * D],
                xt_d[:, g * TPG * D:(g * TPG + n_tiles) * D],
            )

            mt_ps = mt_psum.tile([P, TPG * D], f32, space="PSUM")
            glist = uses[g]
            # first/last use index per tile for start/stop flags
            first_of = {}
            last_of = {}
            for pos, (wi, c, tl) in enumerate(glist):
                first_of.setdefault(tl, pos)
                last_of[tl] = pos
            for pos, (wi, c, tl) in enumerate(glist):
                oh = oh_pool.tile([P, D], bf16, tag="oh")
                uu = u_run + pos
                nc.vector.tensor_scalar(
                    out=oh[:],
                    in0=iota_f[:],
                    scalar1=dstp_sb[:, uu:uu + 1],
                    scalar2=recipe_sb[:, uu:uu + 1],
                    op0=mybir.AluOpType.is_equal,
                    op1=mybir.AluOpType.mult,
                )
                nc.tensor.matmul(
                    out=mt_ps[:, tl * D:(tl + 1) * D],
                    lhsT=stg[wi][:, c * D:(c + 1) * D],
                    rhs=oh[:],
                    start=(pos == first_of[tl]),
                    stop=(pos == last_of[tl]),
                )
            u_run += len(glist)

            mt_sb = mt_pool.tile([P, TPG * D], bf16, tag="mt")
            nc.scalar.copy(mt_sb[:, :n_tiles * D], mt_ps[:, :n_tiles * D])
            z_ps = z_psum.tile([P, TPG * D], f32, space="PSUM")
            nc.tensor.matmul(out=z_ps[:, :n_tiles * D], lhsT=wl_sb[:],
                             rhs=mt_sb[:, :n_tiles * D], start=True, stop=False)
            nc.tensor.matmul(out=z_ps[:, :n_tiles * D], lhsT=wr_sb[:],
                             rhs=xt_sb[:, :n_tiles * D], start=False, stop=True)
            o_sb = out_pool.tile([P, TPG * D], f32, tag="o")
            nc.scalar.activation(
                o_sb[:, :n_tiles * D], z_ps[:, :n_tiles * D],
                mybir.ActivationFunctionType.Relu, bias=b_sb[:, :1], scale=1.0,
            )
            nc.sync.dma_start(
                out_d[:, g * TPG * D:(g * TPG + n_tiles) * D],
                o_sb[:, :n_tiles * D],
            )

    nc.compile()
    return nc


# ---------------------------------------------------------------------------
# Top level
# ---------------------------------------------------------------------------

def _run(inputs, trace=False):
    from concourse import bass_utils

    x = np.ascontiguousarray(np.asarray(inputs["x"], dtype=np.float32))
    ei = np.asarray(inputs["edge_index"], dtype=np.int64)
    w_l = np.asarray(inputs["W_l"], dtype=np.float32)
    b_l = np.ascontiguousarray(np.asarray(inputs["b_l"], dtype=np.float32))
    w_r = np.asarray(inputs["W_r"], dtype=np.float32)
    src, dst = ei[0], ei[1]

    meta, in_parts = _prep(x, src, dst, N_CORES)
    nc = _build(meta)

    n = meta["n"]
    xrows = np.zeros((n + 2, D), dtype=np.float32)
    xrows[1:n + 1] = x
    xrows_bf = _bf16(xrows)
    wl_bf = _bf16(w_l)
    wr_bf = _bf16(w_r)
    b_col = np.ascontiguousarray(b_l.reshape(D, 1), dtype=np.float32)
    in_maps = []
    for m in range(N_CORES):
        part = in_parts[m]
        in_maps.append({
            "xrows": xrows_bf,
            "idxa": _pad_cols(part["idxa"]),
            "idxb": _pad_cols(part["idxb"]),
            "dstp": part["dstp"],
            "recipe": part["recipe"],
            "xt": part["xt"],
            "wl": wl_bf,
            "wr": wr_bf,
            "bias": b_col,
        })

    results = bass_utils.run_bass_kernel_spmd(
        nc, in_maps, core_ids=list(range(N_CORES)), trace=trace
    )

    npc = meta["npc"]
    out = np.empty((n, D), dtype=np.float32)
    for m in range(N_CORES):
        out_t = results.results[m]["outT"]  # [128, nrank] feature-major
        out[m * npc:(m + 1) * npc] = out_t[:, :npc].T
    return out, results


def _pad_cols(a):
    """int16 idx tables can be [128, 0]; the dram tensor is [128, >=1]."""
    if a.shape[1] == 0:
        return np.zeros((128, 1), dtype=np.int16)
    return a


def kernel(**inputs) -> np.ndarray:
    return _run(inputs)[0]


# revision 9
# speedup vs baseline: 1.5994x; 1.0277x over previous
"""GraphSAGE conv layer (PyG SAGEConv, aggr='mean') on 8 Trainium2 NeuronCores.

    out = relu(mean_j(x[src_j]) @ W_l + b_l + x @ W_r)

Sharding: edges are partitioned by destination node across the 8 cores (6250
destination nodes per core); the small 128x128 weights are replicated; each
core keeps a full copy of x in its DRAM so the per-edge source-feature gather
stays local (full-input replication instead of a halo exchange).

Per-core device pipeline (bf16 data, f32 accumulation):
  - Edges are bucketed host-side by (512-node destination group, source-index
    window) and fetched edge-major with bulk `dma_gather` ops. Gather calls
    are striped across all 4 SWDGE queues so all four Q7 core pairs generate
    DMA descriptors concurrently (descriptor generation is the dominant cost).
  - Per-core padding slack in the shared-size index tables is -1, which the
    Q7 gather kernel skips entirely (no descriptors, no DMA).
  - For each 128-edge column the DVE builds a scaled one-hot selector in bf16
    (is_equal against an iota, times 1/deg of the destination) and the PE
    contracts messages^T @ onehot into PSUM, accumulating the feature-major
    per-node mean directly (segment mean == one matmul chain per node tile).
  - Weight-stationary bf16 matmuls add W_l.T @ meanT + W_r.T @ xT, ACT fuses
    bias + ReLU, and the result is stored feature-major; the host transposes
    while assembling the full output.
"""

import math

import numpy as np

N_CORES = 8
D = 128
P = 128
TPG = 4           # node tiles (of 128 nodes) per PSUM group -> 512 wide
WINDOW_ROWS = 32768   # dma_gather int16 index window (rows)
N_QUEUES = 4          # SWDGE queues (Q7 core pairs) to stripe gathers across


def _bf16(a):
    import ml_dtypes
    return np.ascontiguousarray(np.asarray(a).astype(ml_dtypes.bfloat16))


# ---------------------------------------------------------------------------
# Host-side sharding / table prep
# ---------------------------------------------------------------------------

def _prep(x, src, dst, n_cores):
    n, d = x.shape
    assert d == D
    npc = n // n_cores
    assert npc * n_cores == n
    t0 = math.ceil(npc / P)          # node tiles per core
    g0 = math.ceil(t0 / TPG)         # groups per core
    nrank = t0 * P

    # source windows over the device x copy: row 0 and row n+1 are zeros,
    # x row i lives at device row i+1 (50002 rows total).
    n_dev = n + 2
    assert 2 * WINDOW_ROWS >= n_dev, "two windows must cover all of x"
    a_max_src = min(WINDOW_ROWS - 1, n_dev - 1) - 1   # src s -> row s+1
    b_base = max(0, n_dev - WINDOW_ROWS)
    b_pad = min(n_dev - 1, WINDOW_ROWS - 1)           # window-local zero row

    per_core = []
    cnt = np.zeros((n_cores, g0, 2), dtype=np.int64)
    # per (core, g, w, t_loc): first/last edge position inside the call
    starts3 = np.full((n_cores, g0, 2, TPG), -1, dtype=np.int64)
    ends3 = np.full((n_cores, g0, 2, TPG), -1, dtype=np.int64)

    for m in range(n_cores):
        sel = (dst >= m * npc) & (dst < (m + 1) * npc)
        s = src[sel]
        dl = dst[sel] - m * npc
        deg = np.bincount(dl, minlength=npc)
        recip_node = (1.0 / np.maximum(deg, 1)).astype(np.float32)
        tile = dl // P
        grp = tile // TPG
        w = (s > a_max_src).astype(np.int64)
        order = np.lexsort((tile, w, grp))
        s, dl, tile, grp, w = s[order], dl[order], tile[order], grp[order], w[order]
        t_loc = tile - grp * TPG

        for g in range(g0):
            for wi in range(2):
                selgw = (grp == g) & (w == wi)
                cnt[m, g, wi] = selgw.sum()
                if cnt[m, g, wi] == 0:
                    continue
                base = np.nonzero(selgw)[0][0]
                for tl in range(TPG):
                    st = (grp == g) & (w == wi) & (t_loc == tl)
                    c = st.sum()
                    if c == 0:
                        continue
                    first = np.nonzero(st)[0][0] - base
                    starts3[m, g, wi, tl] = first
                    ends3[m, g, wi, tl] = first + c
        per_core.append((s, dl, grp, w, t_loc, recip_node))

    # shared call sizes (in 128-edge columns)
    S = np.ceil(cnt / P).astype(np.int64).max(axis=0)     # [g0, 2]
    colstart = np.zeros((g0, 2), dtype=np.int64)          # per-window cumulative
    acc = [0, 0]
    for g in range(g0):
        for wi in range(2):
            colstart[g, wi] = acc[wi]
            acc[wi] += S[g, wi]
    tot_cols = (acc[0], acc[1])

    # stripe gather calls across SWDGE queues, greedy-balanced by size
    qloads = [0] * N_QUEUES
    queue_of = np.zeros((g0, 2), dtype=np.int64)
    for g in range(g0):
        for wi in range(2):
            if S[g, wi] == 0:
                continue
            q = min(range(N_QUEUES), key=lambda i: qloads[i])
            queue_of[g, wi] = q
            qloads[q] += int(S[g, wi])

    # shared use lists: per (g, w, t_loc) the union (over cores) column range
    uses = [[] for _ in range(g0)]    # per group: list of (w, c, t_loc)
    use_c0 = np.full((g0, 2, TPG), -1, dtype=np.int64)
    use_u0 = np.full((g0, 2, TPG), -1, dtype=np.int64)
    u_tot = 0
    for g in range(g0):
        n_tiles = min(TPG, t0 - g * TPG)
        for tl in range(n_tiles):
            tile_uses = []
            for wi in range(2):
                stm = starts3[:, g, wi, tl]
                enm = ends3[:, g, wi, tl]
                anyc = stm >= 0
                if not anyc.any():
                    continue
                c_lo = int((stm[anyc] // P).min())
                c_hi = int(((enm[anyc] - 1) // P).max())
                use_c0[g, wi, tl] = c_lo
                use_u0[g, wi, tl] = u_tot + len(tile_uses)
                for c in range(c_lo, c_hi + 1):
                    tile_uses.append((wi, c, tl))
            if not tile_uses:
                # keep the PSUM slice defined: one all-masked use
                wi = 0 if S[g, 0] > 0 else 1
                assert S[g, wi] > 0, f"group {g} has no gather columns at all"
                tile_uses.append((wi, 0, tl))
            uses[g].extend(tile_uses)
    u_tot = sum(len(u) for u in uses)

    # global u index per (g, position-in-group-list)
    u_base = np.zeros(g0, dtype=np.int64)
    accu = 0
    for g in range(g0):
        u_base[g] = accu
        accu += len(uses[g])

    # map (g, w, t_loc, c) -> global u (uses within a tile are consecutive cols)
    u_lookup = {}
    for g in range(g0):
        for pos, (wi, c, tl) in enumerate(uses[g]):
            u_lookup[(g, wi, tl, c)] = u_base[g] + pos

    in_parts = []
    for m in range(n_cores):
        s, dl, grp, w, t_loc, recip_node = per_core[m]
        idx_t = [
            np.zeros(int(tot_cols[0]) * P, dtype=np.int16),
            np.full(int(tot_cols[1]) * P, b_pad, dtype=np.int16),
        ]
        dstp = np.full((P, u_tot), -1.0, dtype=np.float32)
        recipe = np.zeros((P, u_tot), dtype=np.float32)

        # per-edge position j inside its (g, w) call
        call_of_edge = grp * 2 + w
        call_sizes = np.bincount(call_of_edge, minlength=g0 * 2)
        call_first = np.concatenate([[0], np.cumsum(call_sizes)])[:-1]
        j = np.arange(len(s)) - call_first[call_of_edge]
        col = j // P
        p = j % P

        # gather index values
        lin = (colstart[grp, w] * P + j).astype(np.int64)
        val_a = (s + 1).astype(np.int64)
        val_b = (s + 1 - b_base).astype(np.int64)
        isa = w == 0
        idx_t[0][lin[isa]] = val_a[isa]
        idx_t[1][lin[~isa]] = val_b[~isa]

        # dstloc / recip tables per use
        u_edge = np.empty(len(s), dtype=np.int64)
        for g in range(g0):
            for wi in range(2):
                for tl in range(TPG):
                    selgwt = (grp == g) & (w == wi) & (t_loc == tl)
                    if not selgwt.any():
                        continue
                    u0 = u_lookup[(g, wi, tl, int(use_c0[g, wi, tl]))]
                    c0 = use_c0[g, wi, tl]
                    u_edge[selgwt] = u0 + (col[selgwt] - c0)
        dstp[p, u_edge] = (dl - (grp * TPG + t_loc) * P).astype(np.float32)
        recipe[p, u_edge] = recip_node[dl]

        # wrap idx tables to [128, n/16]: linear idx i lives at
        # [i % 16, i // 16], replicated 8x down the partition dim (one copy
        # per Q7 core pair).
        def wrap(a):
            if len(a) == 0:
                return np.zeros((128, 0), dtype=np.int16)
            w16 = a.reshape(-1, 16).T
            return np.ascontiguousarray(np.tile(w16, (8, 1)))

        xt = np.zeros((P, nrank), dtype=np.float32)
        xt[:, :npc] = x[m * npc:(m + 1) * npc].T

        in_parts.append({
            "idxa": wrap(idx_t[0]),
            "idxb": wrap(idx_t[1]),
            "dstp": dstp,
            "recipe": recipe,
            "xt": _bf16(xt),
        })

    meta = {
        "n": n, "npc": npc, "t0": t0, "g0": g0, "nrank": nrank,
        "S": S, "colstart": colstart, "tot_cols": tot_cols,
        "uses": uses, "u_tot": u_tot, "b_base": b_base,
        "queue_of": queue_of,
    }
    return meta, in_parts


# ---------------------------------------------------------------------------
# Device kernel builder
# ---------------------------------------------------------------------------

def _build(meta):
    from contextlib import ExitStack

    import concourse.bass as bass  # noqa: F401
    import concourse.mybir as mybir
    import concourse.tile as tile
    from concourse import bacc

    f32 = mybir.dt.float32
    bf16 = mybir.dt.bfloat16
    i16 = mybir.dt.int16
    i32 = mybir.dt.int32
    n = meta["n"]
    t0 = meta["t0"]
    g0 = meta["g0"]
    nrank = meta["nrank"]
    S = meta["S"]
    colstart = meta["colstart"]
    tot_a, tot_b = meta["tot_cols"]
    uses = meta["uses"]
    u_tot = meta["u_tot"]
    b_base = meta["b_base"]
    queue_of = meta["queue_of"]
    n_dev = n + 2
    win_a = min(WINDOW_ROWS, n_dev)
    max_sa = int(max((S[g, 0] for g in range(g0)), default=1)) or 1
    max_sb = int(max((S[g, 1] for g in range(g0)), default=1)) or 1

    nc = bacc.Bacc("TRN2", target_bir_lowering=False, num_swdge_queues=N_QUEUES,
                   dynamic_dma_scratch_size=65536)
    x2_d = nc.dram_tensor("xrows", [n_dev, D], bf16, kind="ExternalInput")
    idxa_d = nc.dram_tensor("idxa", [P, max(tot_a * 8, 1)], i16, kind="ExternalInput")
    idxb_d = nc.dram_tensor("idxb", [P, max(tot_b * 8, 1)], i16, kind="ExternalInput")
    dstp_d = nc.dram_tensor("dstp", [P, u_tot], f32, kind="ExternalInput")
    recipe_d = nc.dram_tensor("recipe", [P, u_tot], f32, kind="ExternalInput")
    xt_d = nc.dram_tensor("xt", [P, nrank], bf16, kind="ExternalInput")
    wl_d = nc.dram_tensor("wl", [D, D], bf16, kind="ExternalInput")
    wr_d = nc.dram_tensor("wr", [D, D], bf16, kind="ExternalInput")
    b_d = nc.dram_tensor("bias", [D, 1], f32, kind="ExternalInput")
    out_d = nc.dram_tensor("outT", [P, nrank], f32, kind="ExternalOutput")

    with ExitStack() as ctx:
        tc = ctx.enter_context(tile.TileContext(nc))
        const = ctx.enter_context(tc.tile_pool(name="const", bufs=1))
        stga_pool = ctx.enter_context(tc.tile_pool(name="stga", bufs=6))
        stgb_pool = ctx.enter_context(tc.tile_pool(name="stgb", bufs=6))
        oh_pool = ctx.enter_context(tc.tile_pool(name="oh", bufs=24))
        mt_pool = ctx.enter_context(tc.tile_pool(name="mt", bufs=3))
        xt_pool = ctx.enter_context(tc.tile_pool(name="xtp", bufs=3))
        out_pool = ctx.enter_context(tc.tile_pool(name="outp", bufs=3))
        mt_psum = ctx.enter_context(tc.tile_pool(name="mtps", bufs=3, space="PSUM"))
        z_psum = ctx.enter_context(tc.tile_pool(name="zps", bufs=3, space="PSUM"))

        iota_i = const.tile([P, D], i32)
        nc.gpsimd.iota(iota_i[:], pattern=[[1, D]], base=0, channel_multiplier=0)
        iota_f = const.tile([P, D], bf16)
        nc.vector.tensor_copy(iota_f[:], iota_i[:])

        idxa_sb = const.tile([P, max(tot_a * 8, 1)], i16)
        nc.sync.dma_start(idxa_sb[:], idxa_d[:, :])
        idxb_sb = const.tile([P, max(tot_b * 8, 1)], i16)
        nc.sync.dma_start(idxb_sb[:], idxb_d[:, :])
        dstp_sb = const.tile([P, u_tot], f32)
        nc.sync.dma_start(dstp_sb[:], dstp_d[:, :])
        recipe_sb = const.tile([P, u_tot], f32)
        nc.sync.dma_start(recipe_sb[:], recipe_d[:, :])
        wl_sb = const.tile([D, D], bf16)
        nc.sync.dma_start(wl_sb[:], wl_d[:, :])
        wr_sb = const.tile([D, D], bf16)
        nc.sync.dma_start(wr_sb[:], wr_d[:, :])
        b_sb = const.tile([D, 1], f32)
        nc.sync.dma_start(b_sb[:], b_d[:, :])

        # zero the gather staging buffers once: trailing -1 indices leave
        # their SBUF rows unwritten, and stale SBUF garbage can decode to
        # NaN (NaN * 0 poisons the PSUM accumulation).
        for pool, sz in ((stga_pool, max_sa), (stgb_pool, max_sb)):
            for _ in range(6):
                t = pool.tile([P, sz * D], bf16, tag="stg_init")
                nc.vector.memset(t[:], 0.0)

        u_run = 0
        for g in range(g0):
            n_tiles = min(TPG, t0 - g * TPG)
            sa, sb = int(S[g, 0]), int(S[g, 1])
            stg = [None, None]
            if sa > 0:
                stg[0] = stga_pool.tile([P, max_sa * D], bf16, tag="stg_init",
                                        name=f"stga_{g}")
                nc.gpsimd.dma_gather(
                    out_ap=stg[0][:, :sa * D].rearrange("p (s e) -> p s e", e=D),
                    in_ap=x2_d[0:win_a, :],
                    idxs_ap=idxa_sb[:, colstart[g, 0] * 8:(colstart[g, 0] + sa) * 8],
                    num_idxs=sa * P,
                    num_idxs_reg=sa * P,
                    elem_size=D,
                    single_packet=False,
                    queue_num=int(queue_of[g, 0]),
                )
            if sb > 0:
                stg[1] = stgb_pool.tile([P, max_sb * D], bf16, tag="stg_init",
                                        name=f"stgb_{g}")
                nc.gpsimd.dma_gather(
                    out_ap=stg[1][:, :sb * D].rearrange("p (s e) -> p s e", e=D),
                    in_ap=x2_d[b_base:n_dev, :],
                    idxs_ap=idxb_sb[:, colstart[g, 1] * 8:(colstart[g, 1] + sb) * 8],
                    num_idxs=sb * P,
                    num_idxs_reg=sb * P,
                    elem_size=D,
                    single_packet=False,
                    queue_num=int(queue_of[g, 1]),
                )

            xt_sb = xt_pool.tile([P, TPG * D], bf16, tag="xt")
            nc.sync.dma_start(
                xt_sb[:, :n_tiles * D],
                xt_d[:, g * TPG * D:(g * TPG + n_tiles) * D],
            )

            mt_ps = mt_psum.tile([P, TPG * D], f32, space="PSUM")
            glist = uses[g]
            # first/last use index per tile for start/stop flags
            first_of = {}
            last_of = {}
            for pos, (wi, c, tl) in enumerate(glist):
                first_of.setdefault(tl, pos)
                last_of[tl] = pos
            for pos, (wi, c, tl) in enumerate(glist):
                oh = oh_pool.tile([P, D], bf16, tag="oh")
                uu = u_run + pos
                nc.vector.tensor_scalar(
                    out=oh[:],
                    in0=iota_f[:],
                    scalar1=dstp_sb[:, uu:uu + 1],
                    scalar2=recipe_sb[:, uu:uu + 1],
                    op0=mybir.AluOpType.is_equal,
                    op1=mybir.AluOpType.mult,
                )
                nc.tensor.matmul(
                    out=mt_ps[:, tl * D:(tl + 1) * D],
                    lhsT=stg[wi][:, c * D:(c + 1) * D],
                    rhs=oh[:],
                    start=(pos == first_of[tl]),
                    stop=(pos == last_of[tl]),
                )
            u_run += len(glist)

            mt_sb = mt_pool.tile([P, TPG * D], bf16, tag="mt")
            nc.scalar.copy(mt_sb[:, :n_tiles * D], mt_ps[:, :n_tiles * D])
            z_ps = z_psum.tile([P, TPG * D], f32, space="PSUM")
            nc.tensor.matmul(out=z_ps[:, :n_tiles * D], lhsT=wl_sb[:],
                             rhs=mt_sb[:, :n_tiles * D], start=True, stop=False)
            nc.tensor.matmul(out=z_ps[:, :n_tiles * D], lhsT=wr_sb[:],
                             rhs=xt_sb[:, :n_tiles * D], start=False, stop=True)
            o_sb = out_pool.tile([P, TPG * D], f32, tag="o")
            nc.scalar.activation(
                o_sb[:, :n_tiles * D], z_ps[:, :n_tiles * D],
                mybir.ActivationFunctionType.Relu, bias=b_sb[:, :1], scale=1.0,
            )
            nc.sync.dma_start(
                out_d[:, g * TPG * D:(g * TPG + n_tiles) * D],
                o_sb[:, :n_tiles * D],
            )

    nc.compile()
    return nc


# ---------------------------------------------------------------------------
# Top level
# ---------------------------------------------------------------------------

def _run(inputs, trace=False):
    from concourse import bass_utils

    x = np.ascontiguousarray(np.asarray(inputs["x"], dtype=np.float32))
    ei = np.asarray(inputs["edge_index"], dtype=np.int64)
    w_l = np.asarray(inputs["W_l"], dtype=np.float32)
    b_l = np.ascontiguousarray(np.asarray(inputs["b_l"], dtype=np.float32))
    w_r = np.asarray(inputs["W_r"], dtype=np.float32)
    src, dst = ei[0], ei[1]

    meta, in_parts = _prep(x, src, dst, N_CORES)
    nc = _build(meta)

    n = meta["n"]
    xrows = np.zeros((n + 2, D), dtype=np.float32)
    xrows[1:n + 1] = x
    xrows_bf = _bf16(xrows)
    wl_bf = _bf16(w_l)
    wr_bf = _bf16(w_r)
    b_col = np.ascontiguousarray(b_l.reshape(D, 1), dtype=np.float32)
    in_maps = []
    for m in range(N_CORES):
        part = in_parts[m]
        in_maps.append({
            "xrows": xrows_bf,
            "idxa": _pad_cols(part["idxa"]),
            "idxb": _pad_cols(part["idxb"]),
            "dstp": part["dstp"],
            "recipe": part["recipe"],
            "xt": part["xt"],
            "wl": wl_bf,
            "wr": wr_bf,
            "bias": b_col,
        })

    results = bass_utils.run_bass_kernel_spmd(
        nc, in_maps, core_ids=list(range(N_CORES)), trace=trace
    )

    npc = meta["npc"]
    out = np.empty((n, D), dtype=np.float32)
    for m in range(N_CORES):
        out_t = results.results[m]["outT"]  # [128, nrank] feature-major
        out[m * npc:(m + 1) * npc] = out_t[:, :npc].T
    return out, results


def _pad_cols(a):
    """int16 idx tables can be [128, 0]; the dram tensor is [128, >=1]."""
    if a.shape[1] == 0:
        return np.zeros((128, 1), dtype=np.int16)
    return a


def kernel(**inputs) -> np.ndarray:
    return _run(inputs)[0]
